# revision 1
# baseline (speedup 1.0000x reference)
"""Trainium2 Bass kernel for nn_BarrierPolicy (CBF-QP safety filter).

Data-parallel over batch: 8 cores x 32768 samples.
Phase A (per 2048-sample tile): load x in "xview" layout, PE-transpose to
"SP2" (stacked pack-2) layout, run the 3-layer MLP + dynamics matmuls on the
tensor engine, transpose results back to xview.
Phase B (full core): Kiwiel variable-fixing active-set solve of the
per-sample box-QP dual (5 iterations + closed-form finish), then
u = clip(-p + lam*g).

Layouts (per tile of 2048 samples):
  xview: SBUF (128, 128): partition r, col 16b+8s0+j <-> sample 256b+2r+s0, coord j
  SP2  : transpose of xview: partition 16b+8s0+j, col r
  padded-pair psum (for 16-row matmul outs, 32-align rule): chunk b=2q+h at
  partitions [32q,32q+16), free-slot h.
  slot : per-sample scalars (128, 16): partition r, col 2b+s0
"""
import numpy as np

B_FULL, N = 262144, 8
NCORES = 8
S = B_FULL // NCORES          # 32768 samples per core
TILE = 2048
NT = S // TILE                # 16 tiles
NSLOT = S // 128              # 256 slot cols per core
T_KIWIEL = 5
LAMCAP = float(2.0 ** 40)
EPS = 1e-12

_CACHE = {}

_CSHAPES = dict(TL2=(128, 128), TL3px=(64, 16), TL3a=(128, 2),
                TDA=(128, 128), TDG=(128, 128), ID128=(128, 128),
                B1v=(128, 1), B2v=(128, 1), B31e=(128, 1), B32e=(128, 1),
                **{f"TL1E{b}": (128, 128) for b in range(8)})


def _consts(W1, b1, W21, b21, W22, b22, W31, b31, W32, b32, A, G):
    f32 = np.float32
    out = {}
    for b in range(8):
        T = np.zeros((128, 128), f32)
        for s0 in range(2):
            T[16 * b + 8 * s0:16 * b + 8 * s0 + 8, 64 * s0:64 * s0 + 64] = W1
        out[f"TL1E{b}"] = T
    TL2 = np.zeros((128, 128), f32)
    for s0 in range(2):
        TL2[64 * s0:64 * s0 + 64, 32 * s0:32 * s0 + 32] = W21
        TL2[64 * s0:64 * s0 + 64, 64 + 32 * s0:64 + 32 * s0 + 32] = W22
    TL3px = np.zeros((64, 16), f32)
    for s0 in range(2):
        TL3px[32 * s0:32 * s0 + 32, 8 * s0:8 * s0 + 8] = W31
    TL3a = np.zeros((128, 2), f32)          # used as slice [64:128)
    for s0 in range(2):
        TL3a[64 + 32 * s0:64 + 32 * s0 + 32, s0:s0 + 1] = W32
    TDA = np.kron(np.eye(16, dtype=f32), A.T.astype(f32))         # out = A x
    TDG = np.kron(np.eye(16, dtype=f32), (-2.0 * G).astype(f32))  # out = -2 G^T x
    ID128 = np.eye(128, dtype=f32)
    B1v = np.concatenate([b1, b1]).reshape(128, 1).astype(f32)
    B2v = np.concatenate([b21, b21, b22, b22]).reshape(128, 1).astype(f32)
    B31e = np.zeros((128, 1), f32)          # bias for padded px evac (3 bases)
    for m in range(3):
        for s0 in range(2):
            B31e[32 * m + 8 * s0:32 * m + 8 * s0 + 8, 0] = b31
    B32e = np.full((128, 1), float(b32[0]), f32)
    out.update(TL2=TL2, TL3px=TL3px, TL3a=TL3a, TDA=TDA, TDG=TDG, ID128=ID128,
               B1v=B1v, B2v=B2v, B31e=B31e, B32e=B32e)
    return out


def build_kernel(nc, tc, x_d, u_d, cds):
    from concourse import mybir
    f32 = mybir.dt.float32
    AL = mybir.AluOpType
    AF = mybir.ActivationFunctionType
    XL = mybir.AxisListType.X

    with (
        tc.tile_pool(name="const", bufs=1) as cpool,
        tc.tile_pool(name="pers", bufs=1) as pers,
        tc.tile_pool(name="work", bufs=2) as work,
        tc.tile_pool(name="psA", bufs=1, space="PSUM") as psA,
        tc.tile_pool(name="psB", bufs=1, space="PSUM") as psB,
    ):
        C = {k: cpool.tile(list(v), f32, tag=k, name=k) for k, v in _CSHAPES.items()}
        for k in _CSHAPES:
            nc.sync.dma_start(C[k][:], cds[k][:])

        FC = S // 16   # 2048 xview cols per core
        def fc_tile(tag):
            return pers.tile([128, FC], f32, tag=tag, name=tag)
        x_xv, p_xv, g_xv = fc_tile("x_xv"), fc_tile("p_xv"), fc_tile("g_xv")
        gt_xv, pt_xv, q_xv = fc_tile("gt_xv"), fc_tile("pt_xv"), fc_tile("q_xv")
        zt_xv, mm_xv = fc_tile("zt_xv"), fc_tile("mm_xv")
        sc1, sc2 = fc_tile("sc1"), fc_tile("sc2")
        def sl_tile(tag):
            return pers.tile([128, NSLOT], f32, tag=tag, name=tag)
        alpha4, lfhx, sxx = sl_tile("alpha4"), sl_tile("lfhx"), sl_tile("sxx")
        c0s, viol, infs = sl_tile("c0s"), sl_tile("viol"), sl_tile("infs")
        nums, dens, lams = sl_tile("nums"), sl_tile("dens"), sl_tile("lams")
        t1s, t2s, nus, bvs = sl_tile("t1s"), sl_tile("t2s"), sl_tile("nus"), sl_tile("bvs")

        # ---------------- Phase A ----------------
        for t in range(NT):
            cs = slice(128 * t, 128 * t + 128)
            ss = slice(16 * t, 16 * t + 16)
            nc.sync.dma_start(
                x_xv[:, cs].rearrange("p (b s j) -> p b s j", b=8, s=2, j=8),
                x_d[t * TILE:(t + 1) * TILE, :].rearrange(
                    "(b r s) j -> r b s j", b=8, r=128, s=2))
            TP = psA.tile([128, 3, 128], f32, tag="TP", name="TP")
            nc.tensor.transpose(TP[:, 0, :], x_xv[:, cs], C["ID128"][:])
            xsp2 = work.tile([128, 128], f32, tag="xsp2", name="xsp2")
            nc.vector.tensor_copy(xsp2[:], TP[:, 0, :])

            h1P = psA.tile([128, 4, 128], f32, tag="h1P", name="h1P")
            x2P = psA.tile([128, 4, 128], f32, tag="x2P", name="x2P")
            LPx = psA.tile([128, 3, 128], f32, tag="LPx", name="LPx")
            alP = psA.tile([128, 3, 128], f32, tag="alP", name="alP")
            h1 = work.tile([128, 8, 128], f32, tag="h1", name="h1")
            x2 = work.tile([128, 8, 128], f32, tag="x2", name="x2")
            pxe = work.tile([128, 3, 128], f32, tag="pxe", name="pxe")
            asle = work.tile([128, 3, 128], f32, tag="asle", name="asle")

            for half in range(2):
                for bi in range(4):
                    b = 4 * half + bi
                    nc.tensor.matmul(h1P[:, bi, :], C[f"TL1E{b}"][:], xsp2[:])
                for bi in range(4):
                    b = 4 * half + bi
                    nc.scalar.activation(h1[:, b, :], h1P[:, bi, :], AF.Relu,
                                         bias=C["B1v"][:])
                for bi in range(4):
                    b = 4 * half + bi
                    nc.tensor.matmul(x2P[:, bi, :], C["TL2"][:], h1[:, b, :])
                for bi in range(4):
                    b = 4 * half + bi
                    nc.scalar.activation(x2[:, b, :], x2P[:, bi, :], AF.Relu,
                                         bias=C["B2v"][:])
                for bi in range(4):
                    b = 4 * half + bi
                    m3, k3 = b % 3, b // 3
                    nc.tensor.matmul(LPx[32 * m3:32 * m3 + 16, k3, :],
                                     C["TL3px"][:], x2[0:64, b, :])
                    nc.tensor.matmul(alP[32 * m3:32 * m3 + 2, k3, :],
                                     C["TL3a"][64:128, :], x2[64:128, b, :])
            nc.gpsimd.memset(pxe[:], 0.0)
            nc.gpsimd.memset(asle[:], 0.0)
            for m in range(3):
                kk = 3 if m < 2 else 2
                nc.vector.tensor_scalar(pxe[32 * m:32 * m + 16, 0:kk, :],
                                        LPx[32 * m:32 * m + 16, 0:kk, :],
                                        C["B31e"][32 * m:32 * m + 16, :], None,
                                        AL.add)
                nc.scalar.activation(asle[32 * m:32 * m + 2, 0:kk, :],
                                     alP[32 * m:32 * m + 2, 0:kk, :], AF.Sigmoid,
                                     bias=C["B32e"][32 * m:32 * m + 2, :])

            nc.tensor.matmul(TP[:, 1, :], C["TDA"][:], xsp2[:])
            nc.tensor.matmul(TP[:, 2, :], C["TDG"][:], xsp2[:])
            axs = work.tile([128, 128], f32, tag="axs", name="axs")
            gsp2 = work.tile([128, 128], f32, tag="gsp2", name="gsp2")
            nc.vector.tensor_copy(axs[:], TP[:, 1, :])
            nc.scalar.activation(gsp2[:], TP[:, 2, :], AF.Copy)

            # transposes back to xview
            trP = psB.tile([128, 2, 128], f32, tag="trP", name="trP")
            nc.tensor.transpose(trP[:, 0, :], gsp2[:], C["ID128"][:])
            nc.tensor.transpose(trP[:, 1, :], axs[:], C["ID128"][:])
            nc.scalar.activation(g_xv[:, cs], trP[:, 0, :], AF.Copy)
            prodA = work.tile([128, 128], f32, tag="prodA", name="prodA")
            nc.vector.scalar_tensor_tensor(prodA[:], trP[:, 1, :], -2.0,
                                           x_xv[:, cs], AL.mult, AL.mult)
            nc.vector.tensor_reduce(lfhx[:, ss],
                                    prodA[:].rearrange("p (c j) -> p c j", j=8),
                                    XL, AL.add)
            sqx = work.tile([128, 128], f32, tag="sqx", name="sqx")
            nc.scalar.activation(sqx[:], x_xv[:, cs], AF.Square)
            nc.vector.tensor_reduce(sxx[:, ss],
                                    sqx[:].rearrange("p (c j) -> p c j", j=8),
                                    XL, AL.add)

            pxtP = psB.tile([128, 3, 128], f32, tag="pxtP", name="pxtP")
            altP = psB.tile([128, 3, 128], f32, tag="altP", name="altP")
            for k in range(3):
                nc.tensor.transpose(pxtP[:, k, :], pxe[:, k, :], C["ID128"][:])
                nc.tensor.transpose(altP[:, k, :], asle[:, k, :], C["ID128"][:])
            for k in range(3):
                nm = 3 if k < 2 else 2
                dstp = p_xv[:, cs].rearrange("p (b s j) -> p b s j",
                                             b=8, s=2, j=8)[:, 3 * k:3 * k + nm, :, :]
                srcp = pxtP[:, k, :].rearrange("p (m g s j) -> p m g s j",
                                               m=4, g=2, s=2, j=8)[:, 0:nm, 0, :, :]
                nc.vector.tensor_copy(dstp, srcp)
                dsta = alpha4[:, ss].rearrange("p (b s) -> p b s",
                                               b=8, s=2)[:, 3 * k:3 * k + nm, :]
                srca = altP[:, k, :].rearrange("p (m g) -> p m g",
                                               m=4, g=32)[:, 0:nm, 0:2]
                nc.vector.tensor_copy(dsta, srca)

        # ---------------- Phase B ----------------
        x3 = lambda ap: ap.rearrange("p (c j) -> p c j", j=8)
        bc = lambda ap: ap.broadcast_to((128, NSLOT, 8))
        V, GP, SC = nc.vector, nc.gpsimd, nc.scalar

        GP.tensor_scalar(alpha4[:], alpha4[:], 4.0, None, AL.mult)
        GP.tensor_scalar(t1s[:], sxx[:], -1.0, 16.0, AL.mult, AL.add)
        V.tensor_tensor(t2s[:], alpha4[:], t1s[:], AL.mult)
        V.tensor_tensor(c0s[:], t2s[:], lfhx[:], AL.add)

        SC.sign(sc1[:], g_xv[:])                                  # sigma
        V.tensor_tensor(pt_xv[:], sc1[:], p_xv[:], AL.mult)       # pt
        GP.tensor_scalar(zt_xv[:], pt_xv[:], -1.0, None, AL.mult)  # zt0
        SC.activation(gt_xv[:], g_xv[:], AF.Abs)
        SC.activation(q_xv[:], g_xv[:], AF.Square)
        V.memset(mm_xv[:], 1.0)

        V.tensor_scalar(sc2[:], p_xv[:], -1.0, 1.0, AL.mult, AL.min)
        V.tensor_scalar(sc2[:], sc2[:], -1.0, None, AL.max)
        V.tensor_tensor(sc2[:], g_xv[:], sc2[:], AL.mult)
        V.tensor_reduce(t1s[:], x3(sc2[:]), XL, AL.add)
        V.tensor_tensor(t1s[:], c0s[:], t1s[:], AL.add)
        GP.tensor_scalar(viol[:], t1s[:], 0.0, None, AL.is_lt)
        V.tensor_reduce(t2s[:], x3(gt_xv[:]), XL, AL.add)
        V.tensor_tensor(t2s[:], c0s[:], t2s[:], AL.add)
        GP.tensor_scalar(infs[:], t2s[:], 0.0, None, AL.is_lt)
        V.tensor_tensor(infs[:], infs[:], viol[:], AL.mult)

        def calc_num_den():
            V.tensor_tensor(sc1[:], gt_xv[:], zt_xv[:], AL.mult)
            V.tensor_reduce(nums[:], x3(sc1[:]), XL, AL.add)
            V.tensor_tensor(nums[:], c0s[:], nums[:], AL.add)
            GP.tensor_tensor(sc2[:], q_xv[:], mm_xv[:], AL.mult)
            V.tensor_reduce(dens[:], x3(sc2[:]), XL, AL.add)

        def calc_lam():
            GP.tensor_scalar(t1s[:], dens[:], EPS, None, AL.add)
            V.reciprocal(t2s[:], t1s[:])
            V.scalar_tensor_tensor(lams[:], nums[:], -1.0, t2s[:], AL.mult, AL.mult)
            V.tensor_tensor(lams[:], lams[:], viol[:], AL.mult)

        calc_num_den()
        for _ in range(T_KIWIEL):
            calc_lam()
            V.tensor_tensor(x3(sc1[:]), bc(lams[:]), x3(gt_xv[:]), AL.mult)
            V.tensor_tensor(sc1[:], sc1[:], pt_xv[:], AL.subtract)   # ur
            V.tensor_scalar(sc2[:], sc1[:], 1.0, -1.0, AL.min, AL.max)
            V.tensor_tensor(sc2[:], gt_xv[:], sc2[:], AL.mult)
            V.tensor_reduce(t1s[:], x3(sc2[:]), XL, AL.add)
            V.tensor_tensor(t1s[:], c0s[:], t1s[:], AL.add)          # c
            GP.tensor_scalar(nus[:], t1s[:], 0.0, None, AL.is_lt)    # needup
            GP.tensor_scalar(bvs[:], nus[:], 2.0, -1.0, AL.mult, AL.add)
            # fix = M * 1{B*ur >= 1}  (== M*(NU*m1 + (1-NU)*m2))
            V.tensor_tensor(x3(sc2[:]), bc(bvs[:]), x3(sc1[:]), AL.mult)
            V.tensor_scalar(sc2[:], sc2[:], 1.0, None, AL.is_ge)
            V.tensor_tensor(sc2[:], sc2[:], mm_xv[:], AL.mult)       # fix
            GP.tensor_tensor(x3(sc1[:]), bc(bvs[:]), x3(zt_xv[:]), AL.subtract)
            V.tensor_tensor(sc1[:], sc2[:], sc1[:], AL.mult)
            V.tensor_tensor(zt_xv[:], zt_xv[:], sc1[:], AL.add)
            GP.tensor_tensor(mm_xv[:], mm_xv[:], sc2[:], AL.subtract)
            calc_num_den()
        calc_lam()
        GP.tensor_scalar(t1s[:], lams[:], -1.0, LAMCAP, AL.mult, AL.add)
        V.tensor_tensor(t1s[:], t1s[:], infs[:], AL.mult)
        V.tensor_tensor(lams[:], lams[:], t1s[:], AL.add)
        V.tensor_tensor(x3(sc1[:]), bc(lams[:]), x3(g_xv[:]), AL.mult)
        V.tensor_tensor(sc1[:], sc1[:], p_xv[:], AL.subtract)
        V.tensor_scalar(sc1[:], sc1[:], 1.0, -1.0, AL.min, AL.max)
        for t in range(NT):
            nc.sync.dma_start(
                u_d[t * TILE:(t + 1) * TILE, :].rearrange(
                    "(b r s) j -> r b s j", b=8, r=128, s=2),
                sc1[:, 128 * t:128 * t + 128].rearrange(
                    "p (b s j) -> p b s j", b=8, s=2, j=8))


def _build():
    from concourse import bacc, mybir
    from concourse import tile as tile_mod
    from concourse._compat import axon_active
    f32 = mybir.dt.float32
    nc = bacc.Bacc("TRN2", target_bir_lowering=False,
                   debug=not axon_active(), num_devices=NCORES)
    x_d = nc.dram_tensor("x", [S, N], f32, kind="ExternalInput").ap()
    u_d = nc.dram_tensor("u", [S, N], f32, kind="ExternalOutput").ap()
    cds = {k: nc.dram_tensor(k, list(v), f32, kind="ExternalInput").ap()
           for k, v in _CSHAPES.items()}
    with tile_mod.TileContext(nc) as tc:
        build_kernel(nc, tc, x_d, u_d, cds)
    nc.compile()
    return nc


def kernel(x, W1, b1, W21, b21, W22, b22, W31, b31, W32, b32, A, G, mean, std):
    from concourse.bass_utils import run_bass_kernel_spmd
    f32 = np.float32
    x = np.asarray(x, f32)
    x0 = (x * np.asarray(std, f32) + np.asarray(mean, f32)).astype(f32)

    consts = _consts(np.asarray(W1, f32), np.asarray(b1, f32), np.asarray(W21, f32),
                     np.asarray(b21, f32), np.asarray(W22, f32), np.asarray(b22, f32),
                     np.asarray(W31, f32), np.asarray(b31, f32), np.asarray(W32, f32),
                     np.asarray(b32, f32), np.asarray(A, f32), np.asarray(G, f32))
    if "nc" not in _CACHE:
        _CACHE["nc"] = _build()
    nc = _CACHE["nc"]

    in_maps = []
    for c in range(NCORES):
        m = {"x": np.ascontiguousarray(x0[c * S:(c + 1) * S])}
        m.update(consts)
        in_maps.append(m)
    res = run_bass_kernel_spmd(nc, in_maps, list(range(NCORES)))
    out = np.concatenate([np.asarray(res.results[c]["u"]) for c in range(NCORES)],
                         axis=0)
    return out.astype(f32)



# revision 2
# speedup vs baseline: 1.5804x; 1.5804x over previous
"""Trainium2 Bass kernel for nn_BarrierPolicy (CBF-QP safety filter).

Data-parallel over batch: 8 cores x 32768 samples.
Per 4-tile group (8192 samples):
  Phase A (per 2048-sample tile): load x in "xview" layout, PE-transpose to
  "SP2" (stacked pack-2) layout, run the 3-layer MLP + dynamics matmuls on
  the tensor engine, transpose results back to xview.
  Phase B (per group, emitted right after the group's 4 tiles so it overlaps
  the next group's Phase A): optimistic-slope Newton solve of the per-sample
  box-QP dual in sign-transformed space (T iterations), then
  u = clip(-p + lam*g).

Newton iteration (per sample, transformed gt=|g|, pt=sign(g)*p, q=g^2):
  ur = lam*gt - pt ; uc = clip(ur) ; c = c0 + sum(gt*uc)
  S  = sum(q * [ur < 1])        (>= true slope for all larger lam -> no
                                 overshoot; all-saturated rows diverge to
                                 +inf lam which reproduces the reference's
                                 infeasible saturation)
  lam = max(lam - c/S, 0)
c0 and a 1e-12 epsilon ride in a 9th reduction lane so the reduces come out
pre-biased.

Layouts (per tile of 2048 samples):
  xview: SBUF (128, 128): partition r, col 16b+8s0+j <-> sample 256b+2r+s0,
  coord j; slot: per-sample scalars (128, 16): partition r, col 2b+s0.
"""
import numpy as np

B_FULL, N = 262144, 8
NCORES = 8
S = B_FULL // NCORES          # 32768 samples per core
TILE = 2048
NT = S // TILE                # 16 tiles
NSLOT = S // 128              # 256 slot cols per core
GROUP = 4                     # tiles per phase-B group
NG = NT // GROUP              # 4 groups
FCG = 128 * GROUP             # 512 xview cols per group
SLG = 16 * GROUP              # 64 slot cols per group
T_NEWTON = 6
EPS = 1e-12

_CACHE = {}

_CSHAPES = dict(TL2=(128, 128), TL3px=(64, 16), TL3a=(128, 2),
                TDA=(128, 128), TDG=(128, 128), ID128=(128, 128),
                B1v=(128, 1), B2v=(128, 1), B31e=(128, 1), B32e=(128, 1),
                **{f"TL1E{b}": (128, 128) for b in range(8)})


def _consts(W1, b1, W21, b21, W22, b22, W31, b31, W32, b32, A, G):
    f32 = np.float32
    out = {}
    for b in range(8):
        T = np.zeros((128, 128), f32)
        for s0 in range(2):
            T[16 * b + 8 * s0:16 * b + 8 * s0 + 8, 64 * s0:64 * s0 + 64] = W1
        out[f"TL1E{b}"] = T
    TL2 = np.zeros((128, 128), f32)
    for s0 in range(2):
        TL2[64 * s0:64 * s0 + 64, 32 * s0:32 * s0 + 32] = W21
        TL2[64 * s0:64 * s0 + 64, 64 + 32 * s0:64 + 32 * s0 + 32] = W22
    TL3px = np.zeros((64, 16), f32)
    for s0 in range(2):
        TL3px[32 * s0:32 * s0 + 32, 8 * s0:8 * s0 + 8] = W31
    TL3a = np.zeros((128, 2), f32)          # used as slice [64:128)
    for s0 in range(2):
        TL3a[64 + 32 * s0:64 + 32 * s0 + 32, s0:s0 + 1] = W32
    TDA = np.kron(np.eye(16, dtype=f32), A.T.astype(f32))         # out = A x
    TDG = np.kron(np.eye(16, dtype=f32), (-2.0 * G).astype(f32))  # out = -2 G^T x
    ID128 = np.eye(128, dtype=f32)
    B1v = np.concatenate([b1, b1]).reshape(128, 1).astype(f32)
    B2v = np.concatenate([b21, b21, b22, b22]).reshape(128, 1).astype(f32)
    B31e = np.zeros((128, 1), f32)          # bias for padded px evac (3 bases)
    for m in range(3):
        for s0 in range(2):
            B31e[32 * m + 8 * s0:32 * m + 8 * s0 + 8, 0] = b31
    B32e = np.full((128, 1), float(b32[0]), f32)
    out.update(TL2=TL2, TL3px=TL3px, TL3a=TL3a, TDA=TDA, TDG=TDG, ID128=ID128,
               B1v=B1v, B2v=B2v, B31e=B31e, B32e=B32e)
    return out


def build_kernel(nc, tc, x_d, u_d, cds):
    from concourse import mybir
    f32 = mybir.dt.float32
    AL = mybir.AluOpType
    AF = mybir.ActivationFunctionType
    XL = mybir.AxisListType.X

    with (
        tc.tile_pool(name="const", bufs=1) as cpool,
        tc.tile_pool(name="pers", bufs=1) as pers,
        tc.tile_pool(name="work", bufs=2) as work,
        tc.tile_pool(name="psA", bufs=1, space="PSUM") as psA,
        tc.tile_pool(name="psB", bufs=1, space="PSUM") as psB,
    ):
        C = {k: cpool.tile(list(v), f32, tag=k, name=k) for k, v in _CSHAPES.items()}
        for k in _CSHAPES:
            nc.sync.dma_start(C[k][:], cds[k][:])

        FC = S // 16   # 2048 xview cols per core
        def fc_tile(tag):
            return pers.tile([128, FC], f32, tag=tag, name=tag)
        x_xv, p_xv, g_xv = fc_tile("x_xv"), fc_tile("p_xv"), fc_tile("g_xv")
        gt_xv, pt_xv, q_xv = fc_tile("gt_xv"), fc_tile("pt_xv"), fc_tile("q_xv")
        def sl_tile(tag, mult=1):
            return pers.tile([128, NSLOT * mult], f32, tag=tag, name=tag)
        alpha4, lfhx, sxx = sl_tile("alpha4"), sl_tile("lfhx"), sl_tile("sxx")
        lams, ccs, sss = sl_tile("lams"), sl_tile("ccs"), sl_tile("sss")
        rs, t1s, t2s = sl_tile("rs"), sl_tile("t1s"), sl_tile("t2s")
        prod9, qm9 = sl_tile("prod9", 9), sl_tile("qm9", 9)
        # persistent double-buffered L3 evac staging (memset once; garbage
        # regions outside the written slices are never read downstream)
        pxeb = [pers.tile([128, 3, 128], f32, tag=f"pxe{i}", name=f"pxe{i}")
                for i in range(2)]
        asleb = [pers.tile([128, 3, 128], f32, tag=f"asle{i}", name=f"asle{i}")
                 for i in range(2)]
        for i in range(2):
            nc.gpsimd.memset(pxeb[i][:], 0.0)
            nc.gpsimd.memset(asleb[i][:], 0.0)
        # epsilon lane of the slope reduce (written once)
        nc.gpsimd.memset(
            qm9[:].rearrange("p (c j) -> p c j", j=9)[:, :, 8:9], EPS)

        x3 = lambda ap: ap.rearrange("p (c j) -> p c j", j=8)
        x9 = lambda ap: ap.rearrange("p (c j) -> p c j", j=9)
        V, GP, SC = nc.vector, nc.gpsimd, nc.scalar

        def phase_a_tile(t):
            cs = slice(128 * t, 128 * t + 128)
            ss = slice(16 * t, 16 * t + 16)
            nc.sync.dma_start(
                x_xv[:, cs].rearrange("p (b s j) -> p b s j", b=8, s=2, j=8),
                x_d[t * TILE:(t + 1) * TILE, :].rearrange(
                    "(b r s) j -> r b s j", b=8, r=128, s=2))
            TP = psA.tile([128, 3, 128], f32, tag="TP", name="TP")
            nc.tensor.transpose(TP[:, 0, :], x_xv[:, cs], C["ID128"][:])
            xsp2 = work.tile([128, 128], f32, tag="xsp2", name="xsp2")
            SC.activation(xsp2[:], TP[:, 0, :], AF.Copy)

            h1P = psA.tile([128, 4, 128], f32, tag="h1P", name="h1P")
            x2P = psA.tile([128, 4, 128], f32, tag="x2P", name="x2P")
            LPx = psA.tile([128, 3, 128], f32, tag="LPx", name="LPx")
            alP = psA.tile([128, 3, 128], f32, tag="alP", name="alP")
            h1 = work.tile([128, 8, 128], f32, tag="h1", name="h1")
            x2 = work.tile([128, 8, 128], f32, tag="x2", name="x2")
            pxe, asle = pxeb[t % 2], asleb[t % 2]

            for half in range(2):
                for bi in range(4):
                    b = 4 * half + bi
                    nc.tensor.matmul(h1P[:, bi, :], C[f"TL1E{b}"][:], xsp2[:])
                for bi in range(4):
                    b = 4 * half + bi
                    SC.activation(h1[:, b, :], h1P[:, bi, :], AF.Relu,
                                  bias=C["B1v"][:])
                for bi in range(4):
                    b = 4 * half + bi
                    nc.tensor.matmul(x2P[:, bi, :], C["TL2"][:], h1[:, b, :])
                for bi in range(4):
                    b = 4 * half + bi
                    SC.activation(x2[:, b, :], x2P[:, bi, :], AF.Relu,
                                  bias=C["B2v"][:])
                for bi in range(4):
                    b = 4 * half + bi
                    m3, k3 = b % 3, b // 3
                    nc.tensor.matmul(LPx[32 * m3:32 * m3 + 16, k3, :],
                                     C["TL3px"][:], x2[0:64, b, :])
                    nc.tensor.matmul(alP[32 * m3:32 * m3 + 2, k3, :],
                                     C["TL3a"][64:128, :], x2[64:128, b, :])
            for m in range(3):
                kk = 3 if m < 2 else 2
                SC.activation(pxe[32 * m:32 * m + 16, 0:kk, :],
                              LPx[32 * m:32 * m + 16, 0:kk, :], AF.Identity,
                              bias=C["B31e"][32 * m:32 * m + 16, :])
                SC.activation(asle[32 * m:32 * m + 2, 0:kk, :],
                              alP[32 * m:32 * m + 2, 0:kk, :], AF.Sigmoid,
                              bias=C["B32e"][32 * m:32 * m + 2, :])

            nc.tensor.matmul(TP[:, 1, :], C["TDA"][:], xsp2[:])
            nc.tensor.matmul(TP[:, 2, :], C["TDG"][:], xsp2[:])
            axs = work.tile([128, 128], f32, tag="axs", name="axs")
            gsp2 = work.tile([128, 128], f32, tag="gsp2", name="gsp2")
            SC.activation(axs[:], TP[:, 1, :], AF.Copy)
            SC.activation(gsp2[:], TP[:, 2, :], AF.Copy)

            # transposes back to xview
            trP = psB.tile([128, 2, 128], f32, tag="trP", name="trP")
            nc.tensor.transpose(trP[:, 0, :], gsp2[:], C["ID128"][:])
            nc.tensor.transpose(trP[:, 1, :], axs[:], C["ID128"][:])
            SC.activation(g_xv[:, cs], trP[:, 0, :], AF.Copy)
            prodA = work.tile([128, 128], f32, tag="prodA", name="prodA")
            V.scalar_tensor_tensor(prodA[:], trP[:, 1, :], -2.0,
                                   x_xv[:, cs], AL.mult, AL.mult)
            V.tensor_reduce(lfhx[:, ss], x3(prodA[:]), XL, AL.add)
            sqx = work.tile([128, 128], f32, tag="sqx", name="sqx")
            SC.activation(sqx[:], x_xv[:, cs], AF.Square)
            V.tensor_reduce(sxx[:, ss], x3(sqx[:]), XL, AL.add)

            pxtP = psB.tile([128, 3, 128], f32, tag="pxtP", name="pxtP")
            altP = psB.tile([128, 3, 128], f32, tag="altP", name="altP")
            for k in range(3):
                nc.tensor.transpose(pxtP[:, k, :], pxe[:, k, :], C["ID128"][:])
                nc.tensor.transpose(altP[:, k, :], asle[:, k, :], C["ID128"][:])
            for k in range(3):
                nm = 3 if k < 2 else 2
                dstp = p_xv[:, cs].rearrange("p (b s j) -> p b s j",
                                             b=8, s=2, j=8)[:, 3 * k:3 * k + nm, :, :]
                srcp = pxtP[:, k, :].rearrange("p (m g s j) -> p m g s j",
                                               m=4, g=2, s=2, j=8)[:, 0:nm, 0, :, :]
                V.tensor_copy(dstp, srcp)
                dsta = alpha4[:, ss].rearrange("p (b s) -> p b s",
                                               b=8, s=2)[:, 3 * k:3 * k + nm, :]
                srca = altP[:, k, :].rearrange("p (m g) -> p m g",
                                               m=4, g=32)[:, 0:nm, 0:2]
                V.tensor_copy(dsta, srca)

        def phase_b_group(g):
            cs = slice(FCG * g, FCG * (g + 1))
            ss = slice(SLG * g, SLG * (g + 1))
            s9 = slice(SLG * 9 * g, SLG * 9 * (g + 1))
            gt, pt, q = gt_xv[:, cs], pt_xv[:, cs], q_xv[:, cs]
            p, gg = p_xv[:, cs], g_xv[:, cs]
            lam, cc, svs = lams[:, ss], ccs[:, ss], sss[:, ss]
            r, d1, d2 = rs[:, ss], t1s[:, ss], t2s[:, ss]
            p9, q9 = x9(prod9[:, s9]), x9(qm9[:, s9])
            bc = lambda ap: ap.broadcast_to((128, SLG, 8))

            sgx = work.tile([128, FCG], f32, tag="sgx", name="sgx")
            ur = work.tile([128, FCG], f32, tag="ur", name="ur")
            uc = work.tile([128, FCG], f32, tag="uc", name="uc")
            mt = work.tile([128, FCG], f32, tag="mt", name="mt")

            # preamble: transform + c0 (c0 lands in prod9's 9th lane)
            SC.sign(sgx[:], gg)
            SC.activation(gt, gg, AF.Abs)
            SC.activation(q, gg, AF.Square)
            V.tensor_tensor(pt, sgx[:], p, AL.mult)
            GP.tensor_scalar(d1, sxx[:, ss], -4.0, 64.0, AL.mult, AL.add)
            GP.tensor_tensor(d2, alpha4[:, ss], d1, AL.mult)
            GP.tensor_tensor(p9[:, :, 8], d2, lfhx[:, ss], AL.add)
            GP.memset(lam, 0.0)

            for _ in range(T_NEWTON):
                V.tensor_tensor(x3(ur[:]), bc(lam), x3(gt), AL.mult)
                V.tensor_tensor(ur[:], ur[:], pt, AL.subtract)
                V.tensor_scalar(uc[:], ur[:], 1.0, -1.0, AL.min, AL.max)
                GP.tensor_tensor(p9[:, :, 0:8], x3(gt), x3(uc[:]), AL.mult)
                V.tensor_scalar(mt[:], ur[:], 1.0, None, AL.is_lt)
                GP.tensor_tensor(q9[:, :, 0:8], x3(q), x3(mt[:]), AL.mult)
                V.tensor_reduce(cc, p9, XL, AL.add)
                V.tensor_reduce(svs, q9, XL, AL.add)
                V.reciprocal(r, svs)
                V.tensor_tensor(d1, cc, r, AL.mult)
                V.scalar_tensor_tensor(d2, d1, -1.0, lam, AL.mult, AL.add)
                V.tensor_scalar(lam, d2, 0.0, None, AL.max)

            # final u = clip(lam*g - p) and store
            V.tensor_tensor(x3(ur[:]), bc(lam), x3(gg), AL.mult)
            V.tensor_tensor(ur[:], ur[:], p, AL.subtract)
            V.tensor_scalar(uc[:], ur[:], 1.0, -1.0, AL.min, AL.max)
            for tt in range(GROUP):
                t = GROUP * g + tt
                nc.sync.dma_start(
                    u_d[t * TILE:(t + 1) * TILE, :].rearrange(
                        "(b r s) j -> r b s j", b=8, r=128, s=2),
                    uc[:, 128 * tt:128 * tt + 128].rearrange(
                        "p (b s j) -> p b s j", b=8, s=2, j=8))

        for g in range(NG):
            for t in range(GROUP * g, GROUP * (g + 1)):
                phase_a_tile(t)
            phase_b_group(g)


def _build():
    from concourse import bacc, mybir
    from concourse import tile as tile_mod
    from concourse._compat import axon_active
    f32 = mybir.dt.float32
    nc = bacc.Bacc("TRN2", target_bir_lowering=False,
                   debug=not axon_active(), num_devices=NCORES)
    x_d = nc.dram_tensor("x", [S, N], f32, kind="ExternalInput").ap()
    u_d = nc.dram_tensor("u", [S, N], f32, kind="ExternalOutput").ap()
    cds = {k: nc.dram_tensor(k, list(v), f32, kind="ExternalInput").ap()
           for k, v in _CSHAPES.items()}
    with tile_mod.TileContext(nc) as tc:
        build_kernel(nc, tc, x_d, u_d, cds)
    nc.compile()
    return nc


def kernel(x, W1, b1, W21, b21, W22, b22, W31, b31, W32, b32, A, G, mean, std):
    from concourse.bass_utils import run_bass_kernel_spmd
    f32 = np.float32
    x = np.asarray(x, f32)
    x0 = (x * np.asarray(std, f32) + np.asarray(mean, f32)).astype(f32)

    consts = _consts(np.asarray(W1, f32), np.asarray(b1, f32), np.asarray(W21, f32),
                     np.asarray(b21, f32), np.asarray(W22, f32), np.asarray(b22, f32),
                     np.asarray(W31, f32), np.asarray(b31, f32), np.asarray(W32, f32),
                     np.asarray(b32, f32), np.asarray(A, f32), np.asarray(G, f32))
    if "nc" not in _CACHE:
        _CACHE["nc"] = _build()
    nc = _CACHE["nc"]

    in_maps = []
    for c in range(NCORES):
        m = {"x": np.ascontiguousarray(x0[c * S:(c + 1) * S])}
        m.update(consts)
        in_maps.append(m)
    res = run_bass_kernel_spmd(nc, in_maps, list(range(NCORES)))
    out = np.concatenate([np.asarray(res.results[c]["u"]) for c in range(NCORES)],
                         axis=0)
    return out.astype(f32)


# revision 10
# speedup vs baseline: 1.7204x; 1.0886x over previous
"""Trainium2 Bass kernel for nn_BarrierPolicy (CBF-QP safety filter).

Data-parallel over batch: 8 cores x 32768 samples, processed in 4 groups of
8192 samples (4 xview tiles of 2048).

Phase A (per group): DMA x tiles, PE-transpose to SP2 layout, then the MLP
with weight-major matmul runs (one Ldweights per layer): L1 uses a single
(16,128) weight against rhs partition slices (K=16), L2 a single block-pair
weight, L3 a single fused (128,18) weight producing px and the alpha logit
together; dynamics matmuls (A x, -2 G^T x) and PE-transposes back to xview.
Activation-engine evacuations are fused into (128,1024) ops.

Phase B (per group, overlaps the next group's Phase A): optimistic-slope
Newton solve of the per-sample box-QP dual in sign-transformed space:
  ur = lam*gt - pt ; uc = clip(ur) ; c = c0 + sum(gt*uc)
  S  = sum(q * [ur < 1])   (upper bound on all future slopes -> monotone
                            convergence from below; all-saturated infeasible
                            rows diverge to huge lam = reference saturation)
  lam = max(lam - c/S, 0)
c0 and a 1e-12 epsilon ride in a 9th reduction lane. Final u = clip(lam*g-p).

Layouts (per tile of 2048 samples):
  xview: SBUF (128, 128): partition r, col 16b+8s0+j <-> sample 256b+2r+s0,
  coord j; slot: per-sample scalars (128, 16): partition r, col 2b+s0.
"""
import numpy as np

B_FULL, N = 262144, 8
NCORES = 8
S = B_FULL // NCORES          # 32768 samples per core
TILE = 2048
NT = S // TILE                # 16 tiles
NSLOT = S // 128              # 256 slot cols per core
GROUP = 4                     # tiles per group
NG = NT // GROUP              # 4 groups
FCG = 128 * GROUP             # 512 xview cols per group
SLG = 16 * GROUP              # 64 slot cols per group
T_NEWTON = 6
EPS = 1e-12

_CACHE = {}

_CSHAPES = dict(TL2=(128, 128), TL3F=(128, 18),
                TDA=(128, 128), TDG=(128, 128), ID128=(128, 128),
                B1v=(128, 1), B2v=(128, 1), B31x=(128, 1), B32e=(128, 1),
                **{f"TL1E{b}": (128, 128) for b in range(8)})


def _consts(W1, b1, W21, b21, W22, b22, W31, b31, W32, b32, A, G):
    f32 = np.float32
    out = {}
    for b in range(8):
        T = np.zeros((128, 128), f32)
        for s0 in range(2):
            T[16 * b + 8 * s0:16 * b + 8 * s0 + 8, 64 * s0:64 * s0 + 64] = W1
        out[f"TL1E{b}"] = T
    TL2 = np.zeros((128, 128), f32)
    for s0 in range(2):
        TL2[64 * s0:64 * s0 + 64, 32 * s0:32 * s0 + 32] = W21
        TL2[64 * s0:64 * s0 + 64, 64 + 32 * s0:64 + 32 * s0 + 32] = W22
    TL3F = np.zeros((128, 18), f32)        # fused px + alpha-logit head
    for s0 in range(2):
        TL3F[32 * s0:32 * s0 + 32, 8 * s0:8 * s0 + 8] = W31
        TL3F[64 + 32 * s0:96 + 32 * s0, 16 + s0:17 + s0] = W32
    TDA = np.kron(np.eye(16, dtype=f32), A.T.astype(f32))         # out = A x
    TDG = np.kron(np.eye(16, dtype=f32), (-2.0 * G).astype(f32))  # out = -2 G^T x
    ID128 = np.eye(128, dtype=f32)
    B1v = np.concatenate([b1, b1]).reshape(128, 1).astype(f32)
    B2v = np.concatenate([b21, b21, b22, b22]).reshape(128, 1).astype(f32)
    B31x = np.zeros((128, 1), f32)         # px bias rows; alpha rows stay 0
    for m in range(4):
        for s0 in range(2):
            B31x[32 * m + 8 * s0:32 * m + 8 * s0 + 8, 0] = b31
    B32e = np.full((128, 1), float(b32[0]), f32)
    out.update(TL2=TL2, TL3F=TL3F, TDA=TDA, TDG=TDG, ID128=ID128,
               B1v=B1v, B2v=B2v, B31x=B31x, B32e=B32e)
    return out


def build_kernel(nc, tc, x_d, u_d, cds):
    from concourse import mybir
    f32 = mybir.dt.float32
    AL = mybir.AluOpType
    AF = mybir.ActivationFunctionType
    XL = mybir.AxisListType.X

    with (
        tc.tile_pool(name="const", bufs=1) as cpool,
        tc.tile_pool(name="pers", bufs=1) as pers,
        tc.tile_pool(name="work", bufs=2) as work,
        tc.tile_pool(name="psT", bufs=2, space="PSUM") as psT,
        tc.tile_pool(name="psW", bufs=1, space="PSUM") as psW,
        tc.tile_pool(name="psL", bufs=1, space="PSUM") as psL,
    ):
        C = {k: cpool.tile(list(v), f32, tag=k, name=k) for k, v in _CSHAPES.items()}
        for k in _CSHAPES:
            nc.sync.dma_start(C[k][:], cds[k][:])

        FC = S // 16   # 2048 xview cols per core
        def fc_tile(tag):
            return pers.tile([128, FC], f32, tag=tag, name=tag)
        x_xv, p_xv, g_xv = fc_tile("x_xv"), fc_tile("p_xv"), fc_tile("g_xv")
        gt_xv, pt_xv, q_xv = fc_tile("gt_xv"), fc_tile("pt_xv"), fc_tile("q_xv")
        def sl_tile(tag, mult=1):
            return pers.tile([128, NSLOT * mult], f32, tag=tag, name=tag)
        alpha4, lfhx, sxx = sl_tile("alpha4"), sl_tile("lfhx"), sl_tile("sxx")
        lams, ccs, sss = sl_tile("lams"), sl_tile("ccs"), sl_tile("sss")
        rs, t1s, t2s = sl_tile("rs"), sl_tile("t1s"), sl_tile("t2s")
        prod9, qm9 = sl_tile("prod9", 9), sl_tile("qm9", 9)
        nc.gpsimd.memset(
            qm9[:].rearrange("p (c j) -> p c j", j=9)[:, :, 8:9], EPS)

        x3 = lambda ap: ap.rearrange("p (c j) -> p c j", j=8)
        x9 = lambda ap: ap.rearrange("p (c j) -> p c j", j=9)
        V, GP, SC = nc.vector, nc.gpsimd, nc.scalar

        def phase_a_group(g):
            csg = slice(FCG * g, FCG * (g + 1))
            ssg = slice(SLG * g, SLG * (g + 1))
            # ---- load + transpose to SP2 ----
            xTP = psT.tile([128, 4, 128], f32, tag="tp", name="xTP")
            for tt in range(GROUP):
                t = GROUP * g + tt
                cs = slice(128 * t, 128 * t + 128)
                nc.sync.dma_start(
                    x_xv[:, cs].rearrange("p (b s j) -> p b s j", b=8, s=2, j=8),
                    x_d[t * TILE:(t + 1) * TILE, :].rearrange(
                        "(b r s) j -> r b s j", b=8, r=128, s=2))
                nc.tensor.transpose(xTP[:, tt, :], x_xv[:, cs], C["ID128"][:])
            xsp2g = work.tile([128, 4, 128], f32, tag="xsp2g", name="xsp2g")
            SC.activation(xsp2g[:], xTP[:], AF.Copy)

            # ---- L1 (weight-major, free=512) ----
            h1 = work.tile([128, 8, 512], f32, tag="h1", name="h1", bufs=1)
            for pair in range(4):
                h1P = psW.tile([128, 2, 512], f32, tag="mmP", name="h1P")
                for s in range(2):
                    b = 2 * pair + s
                    nc.tensor.matmul(h1P[:, s, :], C[f"TL1E{b}"][:], xsp2g[:])
                SC.activation(h1[:, 2 * pair:2 * pair + 2, :], h1P[:], AF.Relu,
                              bias=C["B1v"][:])

            # ---- L2 (single weight) ----
            x2 = work.tile([128, 8, 512], f32, tag="x2", name="x2", bufs=1)
            for pair in range(4):
                x2P = psW.tile([128, 2, 512], f32, tag="mmP", name="x2P")
                for s in range(2):
                    b = 2 * pair + s
                    nc.tensor.matmul(x2P[:, s, :], C["TL2"][:], h1[:, b, :])
                SC.activation(x2[:, 2 * pair:2 * pair + 2, :], x2P[:], AF.Relu,
                              bias=C["B2v"][:])

            # ---- L3 fused px+alpha (single (128,18) weight) ----
            pxalP = psL.tile([128, 3, 512], f32, tag="pxalP", name="pxalP")
            for b in range(8):
                m3, k3 = b % 3, b // 3
                nc.tensor.matmul(pxalP[32 * m3:32 * m3 + 18, k3, :],
                                 C["TL3F"][:], x2[:, b, :])
            pxale = work.tile([128, 3, 512], f32, tag="pxale", name="pxale",
                              bufs=1)
            for m3 in range(3):
                kk = 3 if m3 < 2 else 2
                SC.activation(pxale[32 * m3:32 * m3 + 18, 0:kk, :],
                              pxalP[32 * m3:32 * m3 + 18, 0:kk, :], AF.Identity,
                              bias=C["B31x"][32 * m3:32 * m3 + 18, :])

            # ---- dynamics ----
            dynA = psT.tile([128, 4, 128], f32, tag="tp", name="dynA")
            nc.tensor.matmul(dynA[:], C["TDA"][:], xsp2g[:])
            axsg = work.tile([128, 4, 128], f32, tag="axsg", name="axsg")
            SC.activation(axsg[:], dynA[:], AF.Copy)
            dynG = psT.tile([128, 4, 128], f32, tag="tp", name="dynG")
            nc.tensor.matmul(dynG[:], C["TDG"][:], xsp2g[:])
            gsp2g = work.tile([128, 4, 128], f32, tag="gsp2g", name="gsp2g")
            SC.activation(gsp2g[:], dynG[:], AF.Copy)

            # ---- transposes back to xview ----
            gT = psT.tile([128, 4, 128], f32, tag="tp", name="gT")
            for tt in range(GROUP):
                nc.tensor.transpose(gT[:, tt, :], gsp2g[:, tt, :], C["ID128"][:])
            SC.activation(g_xv[:, csg].rearrange("p (a b) -> p a b", a=4),
                          gT[:], AF.Copy)
            aT = psT.tile([128, 4, 128], f32, tag="tp", name="aT")
            for tt in range(GROUP):
                nc.tensor.transpose(aT[:, tt, :], axsg[:, tt, :], C["ID128"][:])

            # ---- barrier scalars: Lfhx, ||x||^2 ----
            prodA = work.tile([128, 4, 128], f32, tag="prodA", name="prodA")
            V.scalar_tensor_tensor(
                prodA[:], aT[:], -2.0,
                x_xv[:, csg].rearrange("p (a b) -> p a b", a=4),
                AL.mult, AL.mult)
            V.tensor_reduce(lfhx[:, ssg],
                            prodA[:].rearrange("p a (c j) -> p (a c) j", j=8),
                            XL, AL.add)
            sqxg = work.tile([128, 512], f32, tag="sqxg", name="sqxg")
            GP.tensor_tensor(sqxg[:], x_xv[:, csg], x_xv[:, csg], AL.mult)
            V.tensor_reduce(sxx[:, ssg], x3(sqxg[:]), XL, AL.add)

            # ---- px/alpha transposes + extraction ----
            for k3 in range(3):
                nm = 3 if k3 < 2 else 2
                pT = psT.tile([128, 4, 128], f32, tag="tp", name=f"pT{k3}")
                for tt in range(GROUP):
                    nc.tensor.transpose(pT[:, tt, :],
                                        pxale[:, k3, 128 * tt:128 * tt + 128],
                                        C["ID128"][:])
                for tt in range(GROUP):
                    t = GROUP * g + tt
                    cs = slice(128 * t, 128 * t + 128)
                    ss = slice(16 * t, 16 * t + 16)
                    src = pT[:, tt, :].rearrange("p (m h s j) -> p m h s j",
                                                 m=4, h=2, s=2, j=8)
                    dstp = p_xv[:, cs].rearrange("p (b s j) -> p b s j",
                                                 b=8, s=2, j=8)[:, 3 * k3:3 * k3 + nm]
                    V.tensor_copy(dstp, src[:, 0:nm, 0, :, :])
                    dsta = alpha4[:, ss].rearrange("p (b s) -> p b s",
                                                   b=8, s=2)[:, 3 * k3:3 * k3 + nm]
                    V.tensor_copy(dsta, src[:, 0:nm, 1, 0, 0:2])
            SC.activation(alpha4[:, ssg], alpha4[:, ssg], AF.Sigmoid,
                          bias=C["B32e"][:])

        def phase_b_group(g):
            cs = slice(FCG * g, FCG * (g + 1))
            ss = slice(SLG * g, SLG * (g + 1))
            s9 = slice(SLG * 9 * g, SLG * 9 * (g + 1))
            gt, pt, q = gt_xv[:, cs], pt_xv[:, cs], q_xv[:, cs]
            p, gg = p_xv[:, cs], g_xv[:, cs]
            lam, cc, svs = lams[:, ss], ccs[:, ss], sss[:, ss]
            r, d1, d2 = rs[:, ss], t1s[:, ss], t2s[:, ss]
            p9, q9 = x9(prod9[:, s9]), x9(qm9[:, s9])
            bc = lambda ap: ap.broadcast_to((128, SLG, 8))

            sgx = work.tile([128, FCG], f32, tag="sgx", name="sgx")
            ur = work.tile([128, FCG], f32, tag="ur", name="ur")
            uc = work.tile([128, FCG], f32, tag="uc", name="uc")
            mt = work.tile([128, FCG], f32, tag="mt", name="mt")

            # preamble: transform + c0 (c0 lands in prod9's 9th lane)
            SC.sign(sgx[:], gg)
            SC.activation(gt, gg, AF.Abs)
            GP.tensor_tensor(q, gg, gg, AL.mult)
            V.tensor_tensor(pt, sgx[:], p, AL.mult)
            GP.tensor_scalar(d1, sxx[:, ss], -4.0, 64.0, AL.mult, AL.add)
            GP.tensor_tensor(d2, alpha4[:, ss], d1, AL.mult)
            GP.tensor_tensor(p9[:, :, 8], d2, lfhx[:, ss], AL.add)
            GP.memset(lam, 0.0)

            for _ in range(T_NEWTON):
                V.tensor_tensor(x3(ur[:]), bc(lam), x3(gt), AL.mult)
                V.tensor_tensor(ur[:], ur[:], pt, AL.subtract)
                V.tensor_scalar(uc[:], ur[:], 1.0, -1.0, AL.min, AL.max)
                GP.tensor_tensor(p9[:, :, 0:8], x3(gt), x3(uc[:]), AL.mult)
                V.tensor_scalar(mt[:], ur[:], 1.0, None, AL.is_lt)
                GP.tensor_tensor(q9[:, :, 0:8], x3(q), x3(mt[:]), AL.mult)
                V.tensor_reduce(cc, p9, XL, AL.add)
                V.tensor_reduce(svs, q9, XL, AL.add)
                V.reciprocal(r, svs)
                V.tensor_tensor(d1, cc, r, AL.mult)
                V.scalar_tensor_tensor(d2, d1, -1.0, lam, AL.mult, AL.add)
                V.tensor_scalar(lam, d2, 0.0, None, AL.max)

            # final u = clip(lam*g - p) and store
            V.tensor_tensor(x3(ur[:]), bc(lam), x3(gg), AL.mult)
            V.tensor_tensor(ur[:], ur[:], p, AL.subtract)
            V.tensor_scalar(uc[:], ur[:], 1.0, -1.0, AL.min, AL.max)
            for tt in range(GROUP):
                t = GROUP * g + tt
                nc.sync.dma_start(
                    u_d[t * TILE:(t + 1) * TILE, :].rearrange(
                        "(b r s) j -> r b s j", b=8, r=128, s=2),
                    uc[:, 128 * tt:128 * tt + 128].rearrange(
                        "p (b s j) -> p b s j", b=8, s=2, j=8))

        for g in range(NG):
            phase_a_group(g)
            phase_b_group(g)


def _build():
    from concourse import bacc, mybir
    from concourse import tile as tile_mod
    from concourse._compat import axon_active
    f32 = mybir.dt.float32
    nc = bacc.Bacc("TRN2", target_bir_lowering=False,
                   debug=not axon_active(), num_devices=NCORES)
    x_d = nc.dram_tensor("x", [S, N], f32, kind="ExternalInput").ap()
    u_d = nc.dram_tensor("u", [S, N], f32, kind="ExternalOutput").ap()
    cds = {k: nc.dram_tensor(k, list(v), f32, kind="ExternalInput").ap()
           for k, v in _CSHAPES.items()}
    with tile_mod.TileContext(nc) as tc:
        build_kernel(nc, tc, x_d, u_d, cds)
    nc.compile()
    return nc


def kernel(x, W1, b1, W21, b21, W22, b22, W31, b31, W32, b32, A, G, mean, std):
    from concourse.bass_utils import run_bass_kernel_spmd
    f32 = np.float32
    x = np.asarray(x, f32)
    x0 = (x * np.asarray(std, f32) + np.asarray(mean, f32)).astype(f32)

    consts = _consts(np.asarray(W1, f32), np.asarray(b1, f32), np.asarray(W21, f32),
                     np.asarray(b21, f32), np.asarray(W22, f32), np.asarray(b22, f32),
                     np.asarray(W31, f32), np.asarray(b31, f32), np.asarray(W32, f32),
                     np.asarray(b32, f32), np.asarray(A, f32), np.asarray(G, f32))
    if "nc" not in _CACHE:
        _CACHE["nc"] = _build()
    nc = _CACHE["nc"]

    in_maps = []
    for c in range(NCORES):
        m = {"x": np.ascontiguousarray(x0[c * S:(c + 1) * S])}
        m.update(consts)
        in_maps.append(m)
    res = run_bass_kernel_spmd(nc, in_maps, list(range(NCORES)))
    out = np.concatenate([np.asarray(res.results[c]["u"]) for c in range(NCORES)],
                         axis=0)
    return out.astype(f32)


# revision 14
# speedup vs baseline: 2.1857x; 1.2704x over previous
"""Trainium2 Bass kernel for nn_BarrierPolicy (CBF-QP safety filter).

Data-parallel over batch: 8 cores x 32768 samples, processed in 4 groups of
8192 samples (4 xview tiles of 2048).

Phase A (per group): DMA x tiles, PE-transpose to SP2 layout, then the MLP
with weight-major matmul runs (one Ldweights per layer): L1 uses a single
(16,128) weight against rhs partition slices (K=16), L2 a single block-pair
weight, L3 a single fused (128,18) weight producing px and the alpha logit
together; dynamics matmuls (A x, -2 G^T x) and PE-transposes back to xview.
Activation-engine evacuations are fused into (128,1024) ops.

Phase B (per group, overlaps the next group's Phase A): optimistic-slope
Newton solve of the per-sample box-QP dual in sign-transformed space:
  ur = lam*gt - pt ; uc = clip(ur) ; c = c0 + sum(gt*uc)
  S  = sum(q * [ur < 1])   (upper bound on all future slopes -> monotone
                            convergence from below; all-saturated infeasible
                            rows diverge to huge lam = reference saturation)
  lam = max(lam - c/S, 0)
c0 and a 1e-12 epsilon ride in a 9th reduction lane. Final u = clip(lam*g-p).

Layouts (per tile of 2048 samples):
  xview: SBUF (128, 128): partition r, col 16b+8s0+j <-> sample 256b+2r+s0,
  coord j; slot: per-sample scalars (128, 16): partition r, col 2b+s0.
"""
import numpy as np

B_FULL, N = 262144, 8
NCORES = 8
S = B_FULL // NCORES          # 32768 samples per core
TILE = 2048
NT = S // TILE                # 16 tiles
NSLOT = S // 128              # 256 slot cols per core
GROUP = 4                     # tiles per group
NG = NT // GROUP              # 4 groups
FCG = 128 * GROUP             # 512 xview cols per group
SLG = 16 * GROUP              # 64 slot cols per group
T_NEWTON = 6
EPS = 1e-12

_CACHE = {}

_CSHAPES = dict(TL2=(128, 128), TL3F=(128, 18),
                TDA=(128, 128), TDG=(128, 128), ID128=(128, 128), IDr=(128, 128),
                B1v=(128, 1), B2v=(128, 1), B31x=(128, 1), B32e=(128, 1),
                **{f"TL1E{b}": (128, 128) for b in range(8)})


def _consts(W1, b1, W21, b21, W22, b22, W31, b31, W32, b32, A, G):
    f32 = np.float32
    out = {}
    for b in range(8):
        T = np.zeros((128, 128), f32)
        for s0 in range(2):
            T[16 * b + 8 * s0:16 * b + 8 * s0 + 8, 64 * s0:64 * s0 + 64] = W1
        out[f"TL1E{b}"] = T
    TL2 = np.zeros((128, 128), f32)
    for s0 in range(2):
        TL2[64 * s0:64 * s0 + 64, 32 * s0:32 * s0 + 32] = W21
        TL2[64 * s0:64 * s0 + 64, 64 + 32 * s0:64 + 32 * s0 + 32] = W22
    TL3F = np.zeros((128, 18), f32)        # fused px + alpha-logit head
    for s0 in range(2):
        TL3F[32 * s0:32 * s0 + 32, 8 * s0:8 * s0 + 8] = W31
        TL3F[64 + 32 * s0:96 + 32 * s0, 16 + s0:17 + s0] = W32
    TDA = np.kron(np.eye(16, dtype=f32), A.T.astype(f32))         # out = A x
    TDG = np.kron(np.eye(16, dtype=f32), (-2.0 * G).astype(f32))  # out = -2 G^T x
    ID128 = np.eye(128, dtype=f32)
    B1v = np.concatenate([b1, b1]).reshape(128, 1).astype(f32)
    B2v = np.concatenate([b21, b21, b22, b22]).reshape(128, 1).astype(f32)
    B31x = np.zeros((128, 1), f32)         # px bias rows; alpha rows stay 0
    for m in range(4):
        for s0 in range(2):
            B31x[32 * m + 8 * s0:32 * m + 8 * s0 + 8, 0] = b31
    B32e = np.full((128, 1), float(b32[0]), f32)
    out.update(TL2=TL2, TL3F=TL3F, TDA=TDA, TDG=TDG, ID128=ID128, IDr=ID128,
               B1v=B1v, B2v=B2v, B31x=B31x, B32e=B32e)
    return out


def build_kernel(nc, tc, x_d, u_d, cds):
    from concourse import mybir
    f32 = mybir.dt.float32
    f32r = mybir.dt.float32r
    AL = mybir.AluOpType
    AF = mybir.ActivationFunctionType
    XL = mybir.AxisListType.X

    with (
        tc.tile_pool(name="const", bufs=1) as cpool,
        tc.tile_pool(name="pers", bufs=1) as pers,
        tc.tile_pool(name="work", bufs=2) as work,
        tc.tile_pool(name="psT", bufs=2, space="PSUM") as psT,
        tc.tile_pool(name="psW", bufs=1, space="PSUM") as psW,
        tc.tile_pool(name="psL", bufs=1, space="PSUM") as psL,
    ):
        _RDT = {"TL2", "TDA", "TDG", "IDr"} | {f"TL1E{b}" for b in range(8)}
        C = {k: cpool.tile(list(v), f32r if k in _RDT else f32, tag=k, name=k)
             for k, v in _CSHAPES.items()}
        for k in _CSHAPES:
            nc.sync.dma_start(C[k][:], cds[k][:])

        FC = S // 16   # 2048 xview cols per core
        def fc_tile(tag):
            return pers.tile([128, FC], f32, tag=tag, name=tag)
        x_xv, p_xv, g_xv = fc_tile("x_xv"), fc_tile("p_xv"), fc_tile("g_xv")
        gt_xv, pt_xv, q_xv = fc_tile("gt_xv"), fc_tile("pt_xv"), fc_tile("q_xv")
        def sl_tile(tag, mult=1):
            return pers.tile([128, NSLOT * mult], f32, tag=tag, name=tag)
        alpha4, lfhx, sxx = sl_tile("alpha4"), sl_tile("lfhx"), sl_tile("sxx")
        lams, ccs, sss = sl_tile("lams"), sl_tile("ccs"), sl_tile("sss")
        rs, t1s, t2s = sl_tile("rs"), sl_tile("t1s"), sl_tile("t2s")
        prod9, qm9 = sl_tile("prod9", 9), sl_tile("qm9", 9)
        nc.gpsimd.memset(
            qm9[:].rearrange("p (c j) -> p c j", j=9)[:, :, 8:9], EPS)

        x3 = lambda ap: ap.rearrange("p (c j) -> p c j", j=8)
        x9 = lambda ap: ap.rearrange("p (c j) -> p c j", j=9)
        V, GP, SC = nc.vector, nc.gpsimd, nc.scalar

        def phase_a_group(g):
            csg = slice(FCG * g, FCG * (g + 1))
            ssg = slice(SLG * g, SLG * (g + 1))
            # ---- load + transpose to SP2 ----
            xTP = psT.tile([128, 4, 128], f32, tag="tp", name="xTP")
            for tt in range(GROUP):
                t = GROUP * g + tt
                cs = slice(128 * t, 128 * t + 128)
                nc.sync.dma_start(
                    x_xv[:, cs].rearrange("p (b s j) -> p b s j", b=8, s=2, j=8),
                    x_d[t * TILE:(t + 1) * TILE, :].rearrange(
                        "(b r s) j -> r b s j", b=8, r=128, s=2))
                nc.tensor.transpose(xTP[:, tt, :], x_xv[:, cs], C["ID128"][:])
            xsp2g = work.tile([128, 4, 128], f32r, tag="xsp2g", name="xsp2g")
            SC.activation(xsp2g[:], xTP[:], AF.Copy)

            # ---- L1 (weight-major, free=512) ----
            h1 = work.tile([128, 8, 512], f32r, tag="h1", name="h1", bufs=1)
            for pair in range(4):
                h1P = psW.tile([128, 2, 512], f32, tag="mmP", name="h1P")
                for s in range(2):
                    b = 2 * pair + s
                    nc.tensor.matmul(h1P[:, s, :], C[f"TL1E{b}"][:], xsp2g[:])
                SC.activation(h1[:, 2 * pair:2 * pair + 2, :], h1P[:], AF.Relu,
                              bias=C["B1v"][:])

            # ---- L2 (single weight) ----
            x2 = work.tile([128, 8, 512], f32, tag="x2", name="x2", bufs=1)
            for pair in range(4):
                x2P = psW.tile([128, 2, 512], f32, tag="mmP", name="x2P")
                for s in range(2):
                    b = 2 * pair + s
                    nc.tensor.matmul(x2P[:, s, :], C["TL2"][:], h1[:, b, :])
                SC.activation(x2[:, 2 * pair:2 * pair + 2, :], x2P[:], AF.Relu,
                              bias=C["B2v"][:])

            # ---- L3 fused px+alpha (single (128,18) weight) ----
            pxalP = psL.tile([128, 3, 512], f32, tag="pxalP", name="pxalP")
            for b in range(8):
                m3, k3 = b % 3, b // 3
                nc.tensor.matmul(pxalP[32 * m3:32 * m3 + 18, k3, :],
                                 C["TL3F"][:], x2[:, b, :])
            pxale = work.tile([128, 3, 512], f32r, tag="pxale", name="pxale",
                              bufs=1)
            for m3 in range(3):
                kk = 3 if m3 < 2 else 2
                SC.activation(pxale[32 * m3:32 * m3 + 18, 0:kk, :],
                              pxalP[32 * m3:32 * m3 + 18, 0:kk, :], AF.Identity,
                              bias=C["B31x"][32 * m3:32 * m3 + 18, :])

            # ---- dynamics ----
            dynA = psT.tile([128, 4, 128], f32, tag="tp", name="dynA")
            nc.tensor.matmul(dynA[:], C["TDA"][:], xsp2g[:])
            axsg = work.tile([128, 4, 128], f32r, tag="axsg", name="axsg")
            SC.activation(axsg[:], dynA[:], AF.Copy)
            dynG = psT.tile([128, 4, 128], f32, tag="tp", name="dynG")
            nc.tensor.matmul(dynG[:], C["TDG"][:], xsp2g[:])
            gsp2g = work.tile([128, 4, 128], f32r, tag="gsp2g", name="gsp2g")
            SC.activation(gsp2g[:], dynG[:], AF.Copy)

            # ---- transposes back to xview ----
            gT = psT.tile([128, 4, 128], f32r, tag="tp", name="gT")
            for tt in range(GROUP):
                nc.tensor.transpose(gT[:, tt, :], gsp2g[:, tt, :], C["IDr"][:])
            SC.activation(g_xv[:, csg].rearrange("p (a b) -> p a b", a=4),
                          gT[:], AF.Copy)
            aT = psT.tile([128, 4, 128], f32r, tag="tp", name="aT")
            for tt in range(GROUP):
                nc.tensor.transpose(aT[:, tt, :], axsg[:, tt, :], C["IDr"][:])

            # ---- barrier scalars: Lfhx, ||x||^2 ----
            prodA = work.tile([128, 4, 128], f32, tag="prodA", name="prodA")
            V.scalar_tensor_tensor(
                prodA[:], aT[:], -2.0,
                x_xv[:, csg].rearrange("p (a b) -> p a b", a=4),
                AL.mult, AL.mult)
            V.tensor_reduce(lfhx[:, ssg],
                            prodA[:].rearrange("p a (c j) -> p (a c) j", j=8),
                            XL, AL.add)
            sqxg = work.tile([128, 512], f32, tag="sqxg", name="sqxg")
            GP.tensor_tensor(sqxg[:], x_xv[:, csg], x_xv[:, csg], AL.mult)
            V.tensor_reduce(sxx[:, ssg], x3(sqxg[:]), XL, AL.add)

            # ---- px/alpha transposes + extraction ----
            for k3 in range(3):
                nm = 3 if k3 < 2 else 2
                pT = psT.tile([128, 4, 128], f32r, tag="tp", name=f"pT{k3}")
                for tt in range(GROUP):
                    nc.tensor.transpose(pT[:, tt, :],
                                        pxale[:, k3, 128 * tt:128 * tt + 128],
                                        C["IDr"][:])
                src = pT[:].rearrange("p t (m h x) -> p t m h x",
                                      m=4, h=2, x=16)
                dstp = p_xv[:, csg].rearrange(
                    "p (t b x) -> p t b x", t=4, b=8,
                    x=16)[:, :, 3 * k3:3 * k3 + nm]
                V.tensor_copy(dstp, src[:, :, 0:nm, 0, :])
                dsta = alpha4[:, ssg].rearrange(
                    "p (t b s) -> p t b s", t=4, b=8,
                    s=2)[:, :, 3 * k3:3 * k3 + nm]
                V.tensor_copy(dsta, src[:, :, 0:nm, 1, 0:2])
            SC.activation(alpha4[:, ssg], alpha4[:, ssg], AF.Sigmoid,
                          bias=C["B32e"][:])

        def phase_b_group(g):
            cs = slice(FCG * g, FCG * (g + 1))
            ss = slice(SLG * g, SLG * (g + 1))
            s9 = slice(SLG * 9 * g, SLG * 9 * (g + 1))
            gt, pt, q = gt_xv[:, cs], pt_xv[:, cs], q_xv[:, cs]
            p, gg = p_xv[:, cs], g_xv[:, cs]
            lam, cc, svs = lams[:, ss], ccs[:, ss], sss[:, ss]
            r, d1, d2 = rs[:, ss], t1s[:, ss], t2s[:, ss]
            p9, q9 = x9(prod9[:, s9]), x9(qm9[:, s9])
            bc = lambda ap: ap.broadcast_to((128, SLG, 8))

            sgx = work.tile([128, FCG], f32, tag="sgx", name="sgx")
            ur = work.tile([128, FCG], f32, tag="ur", name="ur")
            uc = work.tile([128, FCG], f32, tag="uc", name="uc")
            mt = work.tile([128, FCG], f32, tag="mt", name="mt")

            # preamble: transform + c0 (c0 lands in prod9's 9th lane)
            SC.sign(sgx[:], gg)
            SC.activation(gt, gg, AF.Abs)
            GP.tensor_tensor(q, gg, gg, AL.mult)
            V.tensor_tensor(pt, sgx[:], p, AL.mult)
            GP.tensor_scalar(d1, sxx[:, ss], -4.0, 64.0, AL.mult, AL.add)
            GP.tensor_tensor(d2, alpha4[:, ss], d1, AL.mult)
            GP.tensor_tensor(p9[:, :, 8], d2, lfhx[:, ss], AL.add)
            GP.memset(lam, 0.0)

            for _ in range(T_NEWTON):
                V.tensor_tensor(x3(ur[:]), bc(lam), x3(gt), AL.mult)
                V.tensor_tensor(ur[:], ur[:], pt, AL.subtract)
                V.tensor_scalar(uc[:], ur[:], 1.0, -1.0, AL.min, AL.max)
                GP.tensor_tensor(p9[:, :, 0:8], x3(gt), x3(uc[:]), AL.mult)
                V.tensor_scalar(mt[:], ur[:], 1.0, None, AL.is_lt)
                GP.tensor_tensor(q9[:, :, 0:8], x3(q), x3(mt[:]), AL.mult)
                V.tensor_reduce(cc, p9, XL, AL.add)
                V.tensor_reduce(svs, q9, XL, AL.add)
                V.reciprocal(r, svs)
                V.tensor_tensor(d1, cc, r, AL.mult)
                V.scalar_tensor_tensor(d2, d1, -1.0, lam, AL.mult, AL.add)
                V.tensor_scalar(lam, d2, 0.0, None, AL.max)

            # final u = clip(lam*g - p) and store
            V.tensor_tensor(x3(ur[:]), bc(lam), x3(gg), AL.mult)
            V.tensor_tensor(ur[:], ur[:], p, AL.subtract)
            V.tensor_scalar(uc[:], ur[:], 1.0, -1.0, AL.min, AL.max)
            for tt in range(GROUP):
                t = GROUP * g + tt
                nc.sync.dma_start(
                    u_d[t * TILE:(t + 1) * TILE, :].rearrange(
                        "(b r s) j -> r b s j", b=8, r=128, s=2),
                    uc[:, 128 * tt:128 * tt + 128].rearrange(
                        "p (b s j) -> p b s j", b=8, s=2, j=8))

        for g in range(NG):
            phase_a_group(g)
            phase_b_group(g)


def _build():
    from concourse import bacc, mybir
    from concourse import tile as tile_mod
    from concourse._compat import axon_active
    f32 = mybir.dt.float32
    f32r = mybir.dt.float32r
    nc = bacc.Bacc("TRN2", target_bir_lowering=False,
                   debug=not axon_active(), num_devices=NCORES)
    x_d = nc.dram_tensor("x", [S, N], f32, kind="ExternalInput").ap()
    u_d = nc.dram_tensor("u", [S, N], f32, kind="ExternalOutput").ap()
    _RDT = {"TL2", "TDA", "TDG", "IDr"} | {f"TL1E{b}" for b in range(8)}
    cds = {k: nc.dram_tensor(k, list(v), f32r if k in _RDT else f32,
                             kind="ExternalInput").ap()
           for k, v in _CSHAPES.items()}
    with tile_mod.TileContext(nc) as tc:
        build_kernel(nc, tc, x_d, u_d, cds)
    nc.compile()
    return nc


def kernel(x, W1, b1, W21, b21, W22, b22, W31, b31, W32, b32, A, G, mean, std):
    from concourse.bass_utils import run_bass_kernel_spmd
    f32 = np.float32
    x = np.asarray(x, f32)
    x0 = (x * np.asarray(std, f32) + np.asarray(mean, f32)).astype(f32)

    consts = _consts(np.asarray(W1, f32), np.asarray(b1, f32), np.asarray(W21, f32),
                     np.asarray(b21, f32), np.asarray(W22, f32), np.asarray(b22, f32),
                     np.asarray(W31, f32), np.asarray(b31, f32), np.asarray(W32, f32),
                     np.asarray(b32, f32), np.asarray(A, f32), np.asarray(G, f32))
    if "nc" not in _CACHE:
        _CACHE["nc"] = _build()
    nc = _CACHE["nc"]

    in_maps = []
    for c in range(NCORES):
        m = {"x": np.ascontiguousarray(x0[c * S:(c + 1) * S])}
        m.update(consts)
        in_maps.append(m)
    res = run_bass_kernel_spmd(nc, in_maps, list(range(NCORES)))
    out = np.concatenate([np.asarray(res.results[c]["u"]) for c in range(NCORES)],
                         axis=0)
    return out.astype(f32)


# revision 15
# speedup vs baseline: 2.2109x; 1.0115x over previous
"""Trainium2 Bass kernel for nn_BarrierPolicy (CBF-QP safety filter).

Data-parallel over batch: 8 cores x 32768 samples, processed in 4 groups of
8192 samples (4 xview tiles of 2048).

Phase A (per group): DMA x tiles, PE-transpose to SP2 layout, then the MLP
with weight-major matmul runs (one Ldweights per layer): L1 uses a single
(16,128) weight against rhs partition slices (K=16), L2 a single block-pair
weight, L3 a single fused (128,18) weight producing px and the alpha logit
together; dynamics matmuls (A x, -2 G^T x) and PE-transposes back to xview.
Activation-engine evacuations are fused into (128,1024) ops.

Phase B (per group, overlaps the next group's Phase A): optimistic-slope
Newton solve of the per-sample box-QP dual in sign-transformed space:
  ur = lam*gt - pt ; uc = clip(ur) ; c = c0 + sum(gt*uc)
  S  = sum(q * [ur < 1])   (upper bound on all future slopes -> monotone
                            convergence from below; all-saturated infeasible
                            rows diverge to huge lam = reference saturation)
  lam = max(lam - c/S, 0)
c0 and a 1e-12 epsilon ride in a 9th reduction lane. Final u = clip(lam*g-p).

Layouts (per tile of 2048 samples):
  xview: SBUF (128, 128): partition r, col 16b+8s0+j <-> sample 256b+2r+s0,
  coord j; slot: per-sample scalars (128, 16): partition r, col 2b+s0.
"""
import numpy as np

B_FULL, N = 262144, 8
NCORES = 8
S = B_FULL // NCORES          # 32768 samples per core
TILE = 2048
NT = S // TILE                # 16 tiles
NSLOT = S // 128              # 256 slot cols per core
GROUP = 4                     # tiles per group
NG = NT // GROUP              # 4 groups
FCG = 128 * GROUP             # 512 xview cols per group
SLG = 16 * GROUP              # 64 slot cols per group
T_NEWTON = 6
EPS = 1e-12

_CACHE = {}

_CSHAPES = dict(TL2=(128, 128), TL3F=(128, 18),
                TDA=(128, 128), TDG=(128, 128), ID128=(128, 128), IDr=(128, 128),
                B1v=(128, 1), B2v=(128, 1), B31x=(128, 1), B32e=(128, 1),
                **{f"TL1E{b}": (128, 128) for b in range(8)})


def _consts(W1, b1, W21, b21, W22, b22, W31, b31, W32, b32, A, G):
    f32 = np.float32
    out = {}
    for b in range(8):
        T = np.zeros((128, 128), f32)
        for s0 in range(2):
            T[16 * b + 8 * s0:16 * b + 8 * s0 + 8, 64 * s0:64 * s0 + 64] = W1
        out[f"TL1E{b}"] = T
    TL2 = np.zeros((128, 128), f32)
    for s0 in range(2):
        TL2[64 * s0:64 * s0 + 64, 32 * s0:32 * s0 + 32] = W21
        TL2[64 * s0:64 * s0 + 64, 64 + 32 * s0:64 + 32 * s0 + 32] = W22
    TL3F = np.zeros((128, 18), f32)        # fused px + alpha-logit head
    for s0 in range(2):
        TL3F[32 * s0:32 * s0 + 32, 8 * s0:8 * s0 + 8] = W31
        TL3F[64 + 32 * s0:96 + 32 * s0, 16 + s0:17 + s0] = W32
    TDA = np.kron(np.eye(16, dtype=f32), A.T.astype(f32))         # out = A x
    TDG = np.kron(np.eye(16, dtype=f32), (-2.0 * G).astype(f32))  # out = -2 G^T x
    ID128 = np.eye(128, dtype=f32)
    B1v = np.concatenate([b1, b1]).reshape(128, 1).astype(f32)
    B2v = np.concatenate([b21, b21, b22, b22]).reshape(128, 1).astype(f32)
    B31x = np.zeros((128, 1), f32)         # px bias rows; alpha rows stay 0
    for m in range(4):
        for s0 in range(2):
            B31x[32 * m + 8 * s0:32 * m + 8 * s0 + 8, 0] = b31
    B32e = np.full((128, 1), float(b32[0]), f32)
    out.update(TL2=TL2, TL3F=TL3F, TDA=TDA, TDG=TDG, ID128=ID128, IDr=ID128,
               B1v=B1v, B2v=B2v, B31x=B31x, B32e=B32e)
    return out


def build_kernel(nc, tc, x_d, u_d, cds):
    from concourse import mybir
    f32 = mybir.dt.float32
    f32r = mybir.dt.float32r
    AL = mybir.AluOpType
    AF = mybir.ActivationFunctionType
    XL = mybir.AxisListType.X

    with (
        tc.tile_pool(name="const", bufs=1) as cpool,
        tc.tile_pool(name="pers", bufs=1) as pers,
        tc.tile_pool(name="work", bufs=2) as work,
        tc.tile_pool(name="psT", bufs=2, space="PSUM") as psT,
        tc.tile_pool(name="psW", bufs=1, space="PSUM") as psW,
        tc.tile_pool(name="psL", bufs=1, space="PSUM") as psL,
    ):
        _RDT = {"TL2", "TDA", "TDG", "IDr"} | {f"TL1E{b}" for b in range(8)}
        C = {k: cpool.tile(list(v), f32r if k in _RDT else f32, tag=k, name=k)
             for k, v in _CSHAPES.items()}
        for k in _CSHAPES:
            nc.sync.dma_start(C[k][:], cds[k][:])

        FC = S // 16   # 2048 xview cols per core
        def fc_tile(tag):
            return pers.tile([128, FC], f32, tag=tag, name=tag)
        x_xv, p_xv, g_xv = fc_tile("x_xv"), fc_tile("p_xv"), fc_tile("g_xv")
        gt_xv, pt_xv, q_xv = fc_tile("gt_xv"), fc_tile("pt_xv"), fc_tile("q_xv")
        def sl_tile(tag, mult=1):
            return pers.tile([128, NSLOT * mult], f32, tag=tag, name=tag)
        alpha4, lfhx, sxx = sl_tile("alpha4"), sl_tile("lfhx"), sl_tile("sxx")
        lams, ccs, sss = sl_tile("lams"), sl_tile("ccs"), sl_tile("sss")
        rs, t1s, t2s = sl_tile("rs"), sl_tile("t1s"), sl_tile("t2s")
        prod9, qm9 = sl_tile("prod9", 9), sl_tile("qm9", 9)
        nc.gpsimd.memset(
            qm9[:].rearrange("p (c j) -> p c j", j=9)[:, :, 8:9], EPS)

        x3 = lambda ap: ap.rearrange("p (c j) -> p c j", j=8)
        x9 = lambda ap: ap.rearrange("p (c j) -> p c j", j=9)
        V, GP, SC = nc.vector, nc.gpsimd, nc.scalar

        def phase_a_group(g):
            csg = slice(FCG * g, FCG * (g + 1))
            ssg = slice(SLG * g, SLG * (g + 1))
            # ---- load + transpose to SP2 ----
            nc.sync.dma_start(
                x_xv[:, csg].rearrange("p (c j) -> p c j", j=8),
                x_d[:].rearrange("(p c) j -> p c j", p=128)[:, 64 * g:64 * g + 64, :])
            xTP = psT.tile([128, 4, 128], f32, tag="tp", name="xTP")
            for tt in range(GROUP):
                t = GROUP * g + tt
                cs = slice(128 * t, 128 * t + 128)
                nc.tensor.transpose(xTP[:, tt, :], x_xv[:, cs], C["ID128"][:])
            xsp2g = work.tile([128, 4, 128], f32r, tag="xsp2g", name="xsp2g")
            SC.activation(xsp2g[:], xTP[:], AF.Copy)

            # ---- L1 (weight-major, free=512) ----
            h1 = work.tile([128, 8, 512], f32r, tag="h1", name="h1", bufs=1)
            for pair in range(4):
                h1P = psW.tile([128, 2, 512], f32, tag="mmP", name="h1P")
                for s in range(2):
                    b = 2 * pair + s
                    nc.tensor.matmul(h1P[:, s, :], C[f"TL1E{b}"][:], xsp2g[:])
                SC.activation(h1[:, 2 * pair:2 * pair + 2, :], h1P[:], AF.Relu,
                              bias=C["B1v"][:])

            # ---- L2 (single weight) ----
            x2 = work.tile([128, 8, 512], f32, tag="x2", name="x2", bufs=1)
            for pair in range(4):
                x2P = psW.tile([128, 2, 512], f32, tag="mmP", name="x2P")
                for s in range(2):
                    b = 2 * pair + s
                    nc.tensor.matmul(x2P[:, s, :], C["TL2"][:], h1[:, b, :])
                SC.activation(x2[:, 2 * pair:2 * pair + 2, :], x2P[:], AF.Relu,
                              bias=C["B2v"][:])

            # ---- L3 fused px+alpha (single (128,18) weight) ----
            pxalP = psL.tile([128, 3, 512], f32, tag="pxalP", name="pxalP")
            for b in range(8):
                m3, k3 = b % 3, b // 3
                nc.tensor.matmul(pxalP[32 * m3:32 * m3 + 18, k3, :],
                                 C["TL3F"][:], x2[:, b, :])
            pxale = work.tile([128, 3, 512], f32r, tag="pxale", name="pxale",
                              bufs=1)
            for m3 in range(3):
                kk = 3 if m3 < 2 else 2
                SC.activation(pxale[32 * m3:32 * m3 + 18, 0:kk, :],
                              pxalP[32 * m3:32 * m3 + 18, 0:kk, :], AF.Identity,
                              bias=C["B31x"][32 * m3:32 * m3 + 18, :])

            # ---- dynamics ----
            dynA = psT.tile([128, 4, 128], f32, tag="tp", name="dynA")
            nc.tensor.matmul(dynA[:], C["TDA"][:], xsp2g[:])
            axsg = work.tile([128, 4, 128], f32r, tag="axsg", name="axsg")
            SC.activation(axsg[:], dynA[:], AF.Copy)
            dynG = psT.tile([128, 4, 128], f32, tag="tp", name="dynG")
            nc.tensor.matmul(dynG[:], C["TDG"][:], xsp2g[:])
            gsp2g = work.tile([128, 4, 128], f32r, tag="gsp2g", name="gsp2g")
            SC.activation(gsp2g[:], dynG[:], AF.Copy)

            # ---- transposes back to xview ----
            gT = psT.tile([128, 4, 128], f32r, tag="tp", name="gT")
            for tt in range(GROUP):
                nc.tensor.transpose(gT[:, tt, :], gsp2g[:, tt, :], C["IDr"][:])
            SC.activation(g_xv[:, csg].rearrange("p (a b) -> p a b", a=4),
                          gT[:], AF.Copy)
            aT = psT.tile([128, 4, 128], f32r, tag="tp", name="aT")
            for tt in range(GROUP):
                nc.tensor.transpose(aT[:, tt, :], axsg[:, tt, :], C["IDr"][:])

            # ---- barrier scalars: Lfhx, ||x||^2 ----
            prodA = work.tile([128, 4, 128], f32, tag="prodA", name="prodA")
            V.scalar_tensor_tensor(
                prodA[:], aT[:], -2.0,
                x_xv[:, csg].rearrange("p (a b) -> p a b", a=4),
                AL.mult, AL.mult)
            V.tensor_reduce(lfhx[:, ssg],
                            prodA[:].rearrange("p a (c j) -> p (a c) j", j=8),
                            XL, AL.add)
            sqxg = work.tile([128, 512], f32, tag="sqxg", name="sqxg")
            GP.tensor_tensor(sqxg[:], x_xv[:, csg], x_xv[:, csg], AL.mult)
            V.tensor_reduce(sxx[:, ssg], x3(sqxg[:]), XL, AL.add)

            # ---- px/alpha transposes + extraction ----
            for k3 in range(3):
                nm = 3 if k3 < 2 else 2
                pT = psT.tile([128, 4, 128], f32r, tag="tp", name=f"pT{k3}")
                for tt in range(GROUP):
                    nc.tensor.transpose(pT[:, tt, :],
                                        pxale[:, k3, 128 * tt:128 * tt + 128],
                                        C["IDr"][:])
                src = pT[:].rearrange("p t (m h x) -> p t m h x",
                                      m=4, h=2, x=16)
                dstp = p_xv[:, csg].rearrange(
                    "p (t b x) -> p t b x", t=4, b=8,
                    x=16)[:, :, 3 * k3:3 * k3 + nm]
                V.tensor_copy(dstp, src[:, :, 0:nm, 0, :])
                dsta = alpha4[:, ssg].rearrange(
                    "p (t b s) -> p t b s", t=4, b=8,
                    s=2)[:, :, 3 * k3:3 * k3 + nm]
                V.tensor_copy(dsta, src[:, :, 0:nm, 1, 0:2])
            SC.activation(alpha4[:, ssg], alpha4[:, ssg], AF.Sigmoid,
                          bias=C["B32e"][:])

        def phase_b_group(g):
            cs = slice(FCG * g, FCG * (g + 1))
            ss = slice(SLG * g, SLG * (g + 1))
            s9 = slice(SLG * 9 * g, SLG * 9 * (g + 1))
            gt, pt, q = gt_xv[:, cs], pt_xv[:, cs], q_xv[:, cs]
            p, gg = p_xv[:, cs], g_xv[:, cs]
            lam, cc, svs = lams[:, ss], ccs[:, ss], sss[:, ss]
            r, d1, d2 = rs[:, ss], t1s[:, ss], t2s[:, ss]
            p9, q9 = x9(prod9[:, s9]), x9(qm9[:, s9])
            bc = lambda ap: ap.broadcast_to((128, SLG, 8))

            sgx = work.tile([128, FCG], f32, tag="sgx", name="sgx")
            ur = work.tile([128, FCG], f32, tag="ur", name="ur")
            uc = work.tile([128, FCG], f32, tag="uc", name="uc")
            mt = work.tile([128, FCG], f32, tag="mt", name="mt")

            # preamble: transform + c0 (c0 lands in prod9's 9th lane)
            SC.sign(sgx[:], gg)
            SC.activation(gt, gg, AF.Abs)
            GP.tensor_tensor(q, gg, gg, AL.mult)
            V.tensor_tensor(pt, sgx[:], p, AL.mult)
            GP.tensor_scalar(d1, sxx[:, ss], -4.0, 64.0, AL.mult, AL.add)
            GP.tensor_tensor(d2, alpha4[:, ss], d1, AL.mult)
            GP.tensor_tensor(p9[:, :, 8], d2, lfhx[:, ss], AL.add)
            GP.memset(lam, 0.0)

            for _ in range(T_NEWTON):
                V.tensor_tensor(x3(ur[:]), bc(lam), x3(gt), AL.mult)
                V.tensor_tensor(ur[:], ur[:], pt, AL.subtract)
                V.tensor_scalar(uc[:], ur[:], 1.0, -1.0, AL.min, AL.max)
                GP.tensor_tensor(p9[:, :, 0:8], x3(gt), x3(uc[:]), AL.mult)
                V.tensor_scalar(mt[:], ur[:], 1.0, None, AL.is_lt)
                GP.tensor_tensor(q9[:, :, 0:8], x3(q), x3(mt[:]), AL.mult)
                V.tensor_reduce(cc, p9, XL, AL.add)
                V.tensor_reduce(svs, q9, XL, AL.add)
                V.reciprocal(r, svs)
                V.tensor_tensor(d1, cc, r, AL.mult)
                V.scalar_tensor_tensor(d2, d1, -1.0, lam, AL.mult, AL.add)
                V.tensor_scalar(lam, d2, 0.0, None, AL.max)

            # final u = clip(lam*g - p) and store
            V.tensor_tensor(x3(ur[:]), bc(lam), x3(gg), AL.mult)
            V.tensor_tensor(ur[:], ur[:], p, AL.subtract)
            V.tensor_scalar(uc[:], ur[:], 1.0, -1.0, AL.min, AL.max)
            nc.sync.dma_start(
                u_d[:].rearrange("(p c) j -> p c j", p=128)[:, 64 * g:64 * g + 64, :],
                uc[:].rearrange("p (c j) -> p c j", j=8))

        for g in range(NG):
            phase_a_group(g)
            phase_b_group(g)


def _build():
    from concourse import bacc, mybir
    from concourse import tile as tile_mod
    from concourse._compat import axon_active
    f32 = mybir.dt.float32
    f32r = mybir.dt.float32r
    nc = bacc.Bacc("TRN2", target_bir_lowering=False,
                   debug=not axon_active(), num_devices=NCORES)
    x_d = nc.dram_tensor("x", [S, N], f32, kind="ExternalInput").ap()
    u_d = nc.dram_tensor("u", [S, N], f32, kind="ExternalOutput").ap()
    _RDT = {"TL2", "TDA", "TDG", "IDr"} | {f"TL1E{b}" for b in range(8)}
    cds = {k: nc.dram_tensor(k, list(v), f32r if k in _RDT else f32,
                             kind="ExternalInput").ap()
           for k, v in _CSHAPES.items()}
    with tile_mod.TileContext(nc) as tc:
        build_kernel(nc, tc, x_d, u_d, cds)
    nc.compile()
    return nc


def kernel(x, W1, b1, W21, b21, W22, b22, W31, b31, W32, b32, A, G, mean, std):
    from concourse.bass_utils import run_bass_kernel_spmd
    f32 = np.float32
    x = np.asarray(x, f32)
    x0 = (x * np.asarray(std, f32) + np.asarray(mean, f32)).astype(f32)

    consts = _consts(np.asarray(W1, f32), np.asarray(b1, f32), np.asarray(W21, f32),
                     np.asarray(b21, f32), np.asarray(W22, f32), np.asarray(b22, f32),
                     np.asarray(W31, f32), np.asarray(b31, f32), np.asarray(W32, f32),
                     np.asarray(b32, f32), np.asarray(A, f32), np.asarray(G, f32))
    if "nc" not in _CACHE:
        _CACHE["nc"] = _build()
    nc = _CACHE["nc"]

    in_maps = []
    for c in range(NCORES):
        m = {"x": np.ascontiguousarray(x0[c * S:(c + 1) * S])}
        m.update(consts)
        in_maps.append(m)
    res = run_bass_kernel_spmd(nc, in_maps, list(range(NCORES)))
    out = np.concatenate([np.asarray(res.results[c]["u"]) for c in range(NCORES)],
                         axis=0)
    return out.astype(f32)


# revision 17
# speedup vs baseline: 2.3395x; 1.0582x over previous
"""Trainium2 Bass kernel for nn_BarrierPolicy (CBF-QP safety filter).

Data-parallel over batch: 8 cores x 32768 samples, processed in 4 groups of
8192 samples (4 xview tiles of 2048).

Phase A (per group): DMA x tiles, PE-transpose to SP2 layout, then the MLP
with weight-major matmul runs (one Ldweights per layer): L1 uses a single
(16,128) weight against rhs partition slices (K=16), L2 a single block-pair
weight, L3 a single fused (128,18) weight producing px and the alpha logit
together; dynamics matmuls (A x, -2 G^T x) and PE-transposes back to xview.
Activation-engine evacuations are fused into (128,1024) ops.

Phase B (per group, overlaps the next group's Phase A): optimistic-slope
Newton solve of the per-sample box-QP dual in sign-transformed space:
  ur = lam*gt - pt ; uc = clip(ur) ; c = c0 + sum(gt*uc)
  S  = sum(q * [ur < 1])   (upper bound on all future slopes -> monotone
                            convergence from below; all-saturated infeasible
                            rows diverge to huge lam = reference saturation)
  lam = max(lam - c/S, 0)
c0 and a 1e-12 epsilon ride in a 9th reduction lane. Final u = clip(lam*g-p).

Layouts (per tile of 2048 samples):
  xview: SBUF (128, 128): partition r, col 16b+8s0+j <-> sample 256b+2r+s0,
  coord j; slot: per-sample scalars (128, 16): partition r, col 2b+s0.
"""
import numpy as np

B_FULL, N = 262144, 8
NCORES = 8
S = B_FULL // NCORES          # 32768 samples per core
TILE = 2048
NT = S // TILE                # 16 tiles
NSLOT = S // 128              # 256 slot cols per core
GROUP = 4                     # tiles per group
NG = NT // GROUP              # 4 groups
FCG = 128 * GROUP             # 512 xview cols per group
SLG = 16 * GROUP              # 64 slot cols per group
T_NEWTON = 5
EPS = 1e-12

_CACHE = {}

_CSHAPES = dict(TL2=(128, 128), TL3F=(128, 18),
                TDA=(128, 128), TDG=(128, 128), ID128=(128, 128), IDr=(128, 128),
                B1v=(128, 1), B2v=(128, 1), B31x=(128, 1), B32e=(128, 1),
                **{f"TL1E{b}": (128, 128) for b in range(8)})


def _consts(W1, b1, W21, b21, W22, b22, W31, b31, W32, b32, A, G):
    f32 = np.float32
    out = {}
    for b in range(8):
        T = np.zeros((128, 128), f32)
        for s0 in range(2):
            T[16 * b + 8 * s0:16 * b + 8 * s0 + 8, 64 * s0:64 * s0 + 64] = W1
        out[f"TL1E{b}"] = T
    TL2 = np.zeros((128, 128), f32)
    for s0 in range(2):
        TL2[64 * s0:64 * s0 + 64, 32 * s0:32 * s0 + 32] = W21
        TL2[64 * s0:64 * s0 + 64, 64 + 32 * s0:64 + 32 * s0 + 32] = W22
    TL3F = np.zeros((128, 18), f32)        # fused px + alpha-logit head
    for s0 in range(2):
        TL3F[32 * s0:32 * s0 + 32, 8 * s0:8 * s0 + 8] = W31
        TL3F[64 + 32 * s0:96 + 32 * s0, 16 + s0:17 + s0] = W32
    TDA = np.kron(np.eye(16, dtype=f32), A.T.astype(f32))         # out = A x
    TDG = np.kron(np.eye(16, dtype=f32), (-2.0 * G).astype(f32))  # out = -2 G^T x
    ID128 = np.eye(128, dtype=f32)
    B1v = np.concatenate([b1, b1]).reshape(128, 1).astype(f32)
    B2v = np.concatenate([b21, b21, b22, b22]).reshape(128, 1).astype(f32)
    B31x = np.zeros((128, 1), f32)         # px bias rows; alpha rows stay 0
    for m in range(4):
        for s0 in range(2):
            B31x[32 * m + 8 * s0:32 * m + 8 * s0 + 8, 0] = b31
    B32e = np.full((128, 1), float(b32[0]), f32)
    out.update(TL2=TL2, TL3F=TL3F, TDA=TDA, TDG=TDG, ID128=ID128, IDr=ID128,
               B1v=B1v, B2v=B2v, B31x=B31x, B32e=B32e)
    return out


def build_kernel(nc, tc, x_d, u_d, cds):
    from concourse import mybir
    f32 = mybir.dt.float32
    f32r = mybir.dt.float32r
    AL = mybir.AluOpType
    AF = mybir.ActivationFunctionType
    XL = mybir.AxisListType.X

    with (
        tc.tile_pool(name="const", bufs=1) as cpool,
        tc.tile_pool(name="pers", bufs=1) as pers,
        tc.tile_pool(name="work", bufs=2) as work,
        tc.tile_pool(name="psT", bufs=2, space="PSUM") as psT,
        tc.tile_pool(name="psW", bufs=1, space="PSUM") as psW,
        tc.tile_pool(name="psL", bufs=1, space="PSUM") as psL,
    ):
        _RDT = {"TL2", "TDA", "TDG", "IDr"} | {f"TL1E{b}" for b in range(8)}
        C = {k: cpool.tile(list(v), f32r if k in _RDT else f32, tag=k, name=k)
             for k, v in _CSHAPES.items()}
        for k in _CSHAPES:
            nc.sync.dma_start(C[k][:], cds[k][:])

        FC = S // 16   # 2048 xview cols per core
        def fc_tile(tag):
            return pers.tile([128, FC], f32, tag=tag, name=tag)
        x_xv, p_xv, g_xv = fc_tile("x_xv"), fc_tile("p_xv"), fc_tile("g_xv")
        gt_xv, pt_xv, q_xv = fc_tile("gt_xv"), fc_tile("pt_xv"), fc_tile("q_xv")
        def sl_tile(tag, mult=1):
            return pers.tile([128, NSLOT * mult], f32, tag=tag, name=tag)
        alpha4, lfhx, sxx = sl_tile("alpha4"), sl_tile("lfhx"), sl_tile("sxx")
        lams, ccs, sss = sl_tile("lams"), sl_tile("ccs"), sl_tile("sss")
        rs, t1s, t2s = sl_tile("rs"), sl_tile("t1s"), sl_tile("t2s")
        prod9, qm9 = sl_tile("prod9", 9), sl_tile("qm9", 9)
        nc.gpsimd.memset(
            qm9[:].rearrange("p (c j) -> p c j", j=9)[:, :, 8:9], EPS)

        x3 = lambda ap: ap.rearrange("p (c j) -> p c j", j=8)
        x9 = lambda ap: ap.rearrange("p (c j) -> p c j", j=9)
        V, GP, SC = nc.vector, nc.gpsimd, nc.scalar

        def phase_a_group(g):
            csg = slice(FCG * g, FCG * (g + 1))
            ssg = slice(SLG * g, SLG * (g + 1))
            # ---- load + transpose to SP2 ----
            nc.sync.dma_start(
                x_xv[:, csg].rearrange("p (c j) -> p c j", j=8),
                x_d[:].rearrange("(p c) j -> p c j", p=128)[:, 64 * g:64 * g + 64, :])
            xTP = psT.tile([128, 4, 128], f32, tag="tp", name="xTP")
            for tt in range(GROUP):
                t = GROUP * g + tt
                cs = slice(128 * t, 128 * t + 128)
                nc.tensor.transpose(xTP[:, tt, :], x_xv[:, cs], C["ID128"][:])
            xsp2g = work.tile([128, 4, 128], f32r, tag="xsp2g", name="xsp2g")
            SC.activation(xsp2g[:], xTP[:], AF.Copy)

            # ---- L1 (weight-major, free=512) ----
            h1 = work.tile([128, 8, 512], f32r, tag="h1", name="h1", bufs=1)
            for pair in range(4):
                h1P = psW.tile([128, 2, 512], f32, tag="mmP", name="h1P")
                for s in range(2):
                    b = 2 * pair + s
                    nc.tensor.matmul(h1P[:, s, :], C[f"TL1E{b}"][:], xsp2g[:])
                SC.activation(h1[:, 2 * pair:2 * pair + 2, :], h1P[:], AF.Relu,
                              bias=C["B1v"][:])

            # ---- L2 (single weight) ----
            x2 = work.tile([128, 8, 512], f32, tag="x2", name="x2", bufs=1)
            for pair in range(4):
                x2P = psW.tile([128, 2, 512], f32, tag="mmP", name="x2P")
                for s in range(2):
                    b = 2 * pair + s
                    nc.tensor.matmul(x2P[:, s, :], C["TL2"][:], h1[:, b, :])
                SC.activation(x2[:, 2 * pair:2 * pair + 2, :], x2P[:], AF.Relu,
                              bias=C["B2v"][:])

            # ---- L3 fused px+alpha (single (128,18) weight) ----
            pxalP = psL.tile([128, 3, 512], f32, tag="pxalP", name="pxalP")
            for b in range(8):
                m3, k3 = b % 3, b // 3
                nc.tensor.matmul(pxalP[32 * m3:32 * m3 + 18, k3, :],
                                 C["TL3F"][:], x2[:, b, :])
            pxale = work.tile([128, 3, 512], f32r, tag="pxale", name="pxale",
                              bufs=1)
            for m3 in range(3):
                kk = 3 if m3 < 2 else 2
                SC.activation(pxale[32 * m3:32 * m3 + 18, 0:kk, :],
                              pxalP[32 * m3:32 * m3 + 18, 0:kk, :], AF.Identity,
                              bias=C["B31x"][32 * m3:32 * m3 + 18, :])

            # ---- dynamics ----
            dynA = psT.tile([128, 4, 128], f32, tag="tp", name="dynA")
            nc.tensor.matmul(dynA[:], C["TDA"][:], xsp2g[:])
            axsg = work.tile([128, 4, 128], f32r, tag="axsg", name="axsg")
            SC.activation(axsg[:], dynA[:], AF.Copy)
            dynG = psT.tile([128, 4, 128], f32, tag="tp", name="dynG")
            nc.tensor.matmul(dynG[:], C["TDG"][:], xsp2g[:])
            gsp2g = work.tile([128, 4, 128], f32r, tag="gsp2g", name="gsp2g")
            SC.activation(gsp2g[:], dynG[:], AF.Copy)

            # ---- transposes back to xview ----
            gT = psT.tile([128, 4, 128], f32r, tag="tp", name="gT")
            for tt in range(GROUP):
                nc.tensor.transpose(gT[:, tt, :], gsp2g[:, tt, :], C["IDr"][:])
            SC.activation(g_xv[:, csg].rearrange("p (a b) -> p a b", a=4),
                          gT[:], AF.Copy)
            aT = psT.tile([128, 4, 128], f32r, tag="tp", name="aT")
            for tt in range(GROUP):
                nc.tensor.transpose(aT[:, tt, :], axsg[:, tt, :], C["IDr"][:])

            # ---- barrier scalars: Lfhx, ||x||^2 ----
            prodA = work.tile([128, 4, 128], f32, tag="prodA", name="prodA")
            V.scalar_tensor_tensor(
                prodA[:], aT[:], -2.0,
                x_xv[:, csg].rearrange("p (a b) -> p a b", a=4),
                AL.mult, AL.mult)
            V.tensor_reduce(lfhx[:, ssg],
                            prodA[:].rearrange("p a (c j) -> p (a c) j", j=8),
                            XL, AL.add)
            sqxg = work.tile([128, 512], f32, tag="sqxg", name="sqxg")
            GP.tensor_tensor(sqxg[:], x_xv[:, csg], x_xv[:, csg], AL.mult)
            V.tensor_reduce(sxx[:, ssg], x3(sqxg[:]), XL, AL.add)

            # ---- px/alpha transposes + extraction ----
            for k3 in range(3):
                nm = 3 if k3 < 2 else 2
                pT = psT.tile([128, 4, 128], f32r, tag="tp", name=f"pT{k3}")
                for tt in range(GROUP):
                    nc.tensor.transpose(pT[:, tt, :],
                                        pxale[:, k3, 128 * tt:128 * tt + 128],
                                        C["IDr"][:])
                src = pT[:].rearrange("p t (m h x) -> p t m h x",
                                      m=4, h=2, x=16)
                dstp = p_xv[:, csg].rearrange(
                    "p (t b x) -> p t b x", t=4, b=8,
                    x=16)[:, :, 3 * k3:3 * k3 + nm]
                V.tensor_copy(dstp, src[:, :, 0:nm, 0, :])
                dsta = alpha4[:, ssg].rearrange(
                    "p (t b s) -> p t b s", t=4, b=8,
                    s=2)[:, :, 3 * k3:3 * k3 + nm]
                V.tensor_copy(dsta, src[:, :, 0:nm, 1, 0:2])
            SC.activation(alpha4[:, ssg], alpha4[:, ssg], AF.Sigmoid,
                          bias=C["B32e"][:])

        def phase_b_group(g):
            cs = slice(FCG * g, FCG * (g + 1))
            ss = slice(SLG * g, SLG * (g + 1))
            s9 = slice(SLG * 9 * g, SLG * 9 * (g + 1))
            gt, pt, q = gt_xv[:, cs], pt_xv[:, cs], q_xv[:, cs]
            p, gg = p_xv[:, cs], g_xv[:, cs]
            lam, cc, svs = lams[:, ss], ccs[:, ss], sss[:, ss]
            r, d1, d2 = rs[:, ss], t1s[:, ss], t2s[:, ss]
            p9, q9 = x9(prod9[:, s9]), x9(qm9[:, s9])
            bc = lambda ap: ap.broadcast_to((128, SLG, 8))

            sgx = work.tile([128, FCG], f32, tag="sgx", name="sgx")
            ur = work.tile([128, FCG], f32, tag="ur", name="ur")
            uc = work.tile([128, FCG], f32, tag="uc", name="uc")
            mt = work.tile([128, FCG], f32, tag="mt", name="mt")

            # preamble: transform + c0 (c0 lands in prod9's 9th lane)
            SC.sign(sgx[:], gg)
            SC.activation(gt, gg, AF.Abs)
            GP.tensor_tensor(q, gg, gg, AL.mult)
            V.tensor_tensor(pt, sgx[:], p, AL.mult)
            GP.tensor_scalar(d1, sxx[:, ss], -4.0, 64.0, AL.mult, AL.add)
            GP.tensor_tensor(d2, alpha4[:, ss], d1, AL.mult)
            GP.tensor_tensor(p9[:, :, 8], d2, lfhx[:, ss], AL.add)
            GP.memset(lam, 0.0)

            for _ in range(T_NEWTON):
                V.tensor_tensor(x3(ur[:]), bc(lam), x3(gt), AL.mult)
                V.tensor_tensor(ur[:], ur[:], pt, AL.subtract)
                V.tensor_scalar(uc[:], ur[:], 1.0, -1.0, AL.min, AL.max)
                GP.tensor_tensor(p9[:, :, 0:8], x3(gt), x3(uc[:]), AL.mult)
                V.tensor_scalar(mt[:], ur[:], 1.0, None, AL.is_lt)
                GP.tensor_tensor(q9[:, :, 0:8], x3(q), x3(mt[:]), AL.mult)
                V.tensor_reduce(cc, p9, XL, AL.add)
                V.tensor_reduce(svs, q9, XL, AL.add)
                V.reciprocal(r, svs)
                V.tensor_tensor(d1, cc, r, AL.mult)
                V.scalar_tensor_tensor(d2, d1, -1.0, lam, AL.mult, AL.add)
                V.tensor_scalar(lam, d2, 0.0, None, AL.max)

            # final u = clip(lam*g - p) and store
            V.tensor_tensor(x3(ur[:]), bc(lam), x3(gg), AL.mult)
            V.tensor_tensor(ur[:], ur[:], p, AL.subtract)
            V.tensor_scalar(uc[:], ur[:], 1.0, -1.0, AL.min, AL.max)
            nc.sync.dma_start(
                u_d[:].rearrange("(p c) j -> p c j", p=128)[:, 64 * g:64 * g + 64, :],
                uc[:].rearrange("p (c j) -> p c j", j=8))

        for g in range(NG):
            phase_a_group(g)
            phase_b_group(g)


def _build():
    from concourse import bacc, mybir
    from concourse import tile as tile_mod
    from concourse._compat import axon_active
    f32 = mybir.dt.float32
    f32r = mybir.dt.float32r
    nc = bacc.Bacc("TRN2", target_bir_lowering=False,
                   debug=not axon_active(), num_devices=NCORES)
    x_d = nc.dram_tensor("x", [S, N], f32, kind="ExternalInput").ap()
    u_d = nc.dram_tensor("u", [S, N], f32, kind="ExternalOutput").ap()
    _RDT = {"TL2", "TDA", "TDG", "IDr"} | {f"TL1E{b}" for b in range(8)}
    cds = {k: nc.dram_tensor(k, list(v), f32r if k in _RDT else f32,
                             kind="ExternalInput").ap()
           for k, v in _CSHAPES.items()}
    with tile_mod.TileContext(nc) as tc:
        build_kernel(nc, tc, x_d, u_d, cds)
    nc.compile()
    return nc


def kernel(x, W1, b1, W21, b21, W22, b22, W31, b31, W32, b32, A, G, mean, std):
    from concourse.bass_utils import run_bass_kernel_spmd
    f32 = np.float32
    x = np.asarray(x, f32)
    x0 = (x * np.asarray(std, f32) + np.asarray(mean, f32)).astype(f32)

    consts = _consts(np.asarray(W1, f32), np.asarray(b1, f32), np.asarray(W21, f32),
                     np.asarray(b21, f32), np.asarray(W22, f32), np.asarray(b22, f32),
                     np.asarray(W31, f32), np.asarray(b31, f32), np.asarray(W32, f32),
                     np.asarray(b32, f32), np.asarray(A, f32), np.asarray(G, f32))
    if "nc" not in _CACHE:
        _CACHE["nc"] = _build()
    nc = _CACHE["nc"]

    in_maps = []
    for c in range(NCORES):
        m = {"x": np.ascontiguousarray(x0[c * S:(c + 1) * S])}
        m.update(consts)
        in_maps.append(m)
    res = run_bass_kernel_spmd(nc, in_maps, list(range(NCORES)))
    out = np.concatenate([np.asarray(res.results[c]["u"]) for c in range(NCORES)],
                         axis=0)
    return out.astype(f32)
